# revision 16
# baseline (speedup 1.0000x reference)
"""Trainium2 Bass kernel for nn_EnsembleModel_61718680044080 (nms_detection).

kernel(**inputs) -> [2048, 20] float32 (fused rec lists).

Plan (8 NeuronCores, SPMD):
  Kernel 1 (heavy):
    - prior^T = user_personalities @ X^T on PE (fp32), staged via internal DRAM.
    - sim scores = prior @ user_ratings, item-sharded: each core scores its 2500-item
      slice for ALL 2048 rows (fp32 PE matmuls), then per-row top-24 of each 1250-wide
      half on the vector engine (max8/max_index/match_replace rounds) -> candidate
      values + global item ids.
    - top/mid decoder paths, batch-sharded: each core computes top_sub [256,2000] and
      mid_sub [256,5000] for its own 256 rows and emits per-row top-24 positions
      (mid in two 2500-wide halves with values, merged on host).
  Host glue (numpy only): merge candidate lists by value, map positions through
    top_map/mid_map, softmax logits, and the Gumbel noise consumed by
    jax.random.categorical in the reference (replicated bit-exactly in numpy:
    threefry2x32 splits + XLA Philox4x32-10 bit stream, rbg PRNG impl).
  Kernel 2 (fusion): batch-sharded exact replication of the reference's greedy
    set-intersection fusion + 20-step sampling scan, fully vectorized on the DVE.
"""
import numpy as np
import concourse.bass as bass
import concourse.bacc as bacc
import concourse.mybir as mybir
from concourse import tile
from concourse.bass_utils import run_bass_kernel_spmd

F32 = mybir.dt.float32
U32 = mybir.dt.uint32
ALU = mybir.AluOpType
AX = mybir.AxisListType

B = 2048          # batch
PD = 64           # p_dim
LAT = 128         # latent
NU = 2048         # users (padded 2000 -> 2048)
NT = 2000         # n_top
NM = 5000         # n_mid
MH = 2500         # mid half width
SLICE = 2500      # sim items per core
HALF = 1250       # sim half width
NCAND = 24
NEG = -3.0e38
K = 20
NCORES = 8

# ----------------------------------------------------------------------------
# numpy replication of this environment's jax.random (rbg impl):
# threefry2x32 for key/split, XLA Philox4x32-10 for bits.
# ----------------------------------------------------------------------------
_ROT = ((13, 15, 26, 6), (17, 29, 16, 24))


def _threefry2x32(k0, k1, x0, x1):
    k0 = np.asarray(k0, np.uint32); k1 = np.asarray(k1, np.uint32)
    x0 = np.asarray(x0, np.uint32).copy(); x1 = np.asarray(x1, np.uint32).copy()
    with np.errstate(over="ignore"):
        ks2 = k0 ^ k1 ^ np.uint32(0x1BD11BDA)
        ks = (k0, k1, ks2)
        x0 = x0 + ks[0]; x1 = x1 + ks[1]
        for i in range(5):
            for r in _ROT[i % 2]:
                x0 = x0 + x1
                x1 = ((x1 << np.uint32(r)) | (x1 >> np.uint32(32 - r))) ^ x0
            x0 = x0 + ks[(i + 1) % 3]
            x1 = x1 + ks[(i + 2) % 3] + np.uint32(i + 1)
    return x0, x1


_M0 = np.uint64(0xD2511F53); _M1 = np.uint64(0xCD9E8D57)
_W0 = np.uint32(0x9E3779B9); _W1 = np.uint32(0xBB67AE85)
_MASK64 = np.uint64(0xFFFFFFFF)


def _philox_block(c0, c1, c2, c3, k0, k1):
    c = [np.asarray(c0, np.uint32).copy(), np.asarray(c1, np.uint32).copy(),
         np.asarray(c2, np.uint32).copy(), np.asarray(c3, np.uint32).copy()]
    k0 = np.asarray(k0, np.uint32).copy(); k1 = np.asarray(k1, np.uint32).copy()
    with np.errstate(over="ignore"):
        for _ in range(10):
            p0 = _M0 * c[0].astype(np.uint64)
            p1 = _M1 * c[2].astype(np.uint64)
            hi0 = (p0 >> np.uint64(32)).astype(np.uint32); lo0 = (p0 & _MASK64).astype(np.uint32)
            hi1 = (p1 >> np.uint64(32)).astype(np.uint32); lo1 = (p1 & _MASK64).astype(np.uint32)
            c = [hi1 ^ c[1] ^ k0, lo1, hi0 ^ c[3] ^ k1, lo0]
            k0 = k0 + _W0; k1 = k1 + _W1
    return c


def _philox_stream(k0, k1, n_u32):
    nblk = (n_u32 + 3) // 4
    c64 = (np.uint64(k1) << np.uint64(32)) | np.uint64(k0)
    with np.errstate(over="ignore"):
        cs = c64 + np.arange(nblk, dtype=np.uint64)
    clo = (cs & _MASK64).astype(np.uint32)
    chi = (cs >> np.uint64(32)).astype(np.uint32)
    w = _philox_block(clo, chi, np.full(nblk, k0, np.uint32), np.full(nblk, k1, np.uint32),
                      np.full(nblk, k0, np.uint32), np.full(nblk, k1, np.uint32))
    out = np.empty((nblk, 4), np.uint32)
    for j in range(4):
        out[:, j] = w[j]
    return out.reshape(-1)[:n_u32]


def _gumbel_G(Bn=B, Kn=K, seed=42):
    """G[b,t,c]: gumbel noise consumed by the reference's vmapped categorical scan.
    Under vmap, the rbg impl draws each step's whole [B,3] block from ROW 0's sub key."""
    kk = np.array([(seed >> 32) & 0xFFFFFFFF, seed & 0xFFFFFFFF], np.uint32)
    y0, y1 = _threefry2x32(kk[0], kk[1], np.zeros(Bn, np.uint32), np.arange(Bn, dtype=np.uint32))
    a0, b0 = y0[0], y1[0]               # row 0's key words
    G = np.empty((Bn, Kn, 3), np.float32)
    tiny = np.float32(np.finfo(np.float32).tiny)
    onef = np.float32(1.0)
    for t in range(Kn):
        n0, n1 = _threefry2x32(a0, b0, 0, 0)
        s0, s1 = _threefry2x32(a0, b0, 0, 1)
        bits = _philox_stream(np.uint32(s0), np.uint32(s1), Bn * 3)
        u = ((bits >> np.uint32(9)) | np.uint32(0x3F800000)).view(np.float32) - onef
        u = u * (onef - tiny) + tiny
        np.maximum(u, tiny, out=u)
        G[:, t, :] = (-np.log(-np.log(u))).reshape(Bn, 3)
        a0, b0 = n0, n1
    return G


# ----------------------------------------------------------------------------
# Kernel 1 (heavy): prior^T, sim scoring + top-24, top/mid top-24 positions
# ----------------------------------------------------------------------------
def _topk24(nc, pool, s, w):
    v = pool.tile([128, NCAND], F32, tag="tk_v", name="tk_v")
    ix = pool.tile([128, NCAND], U32, tag="tk_i", name="tk_i")
    s2 = pool.tile([128, 2560], F32, tag="tk_s2", name="tk_s2")
    s3 = pool.tile([128, 2560], F32, tag="tk_s3", name="tk_s3")
    nc.vector.max(v[:, 0:8], s[:])
    nc.vector.max_index(ix[:, 0:8], v[:, 0:8], s[:])
    nc.vector.match_replace(s2[:, :w], v[:, 0:8], s[:], NEG)
    nc.vector.max(v[:, 8:16], s2[:, :w])
    nc.vector.max_index(ix[:, 8:16], v[:, 8:16], s2[:, :w])
    nc.vector.match_replace(s3[:, :w], v[:, 8:16], s2[:, :w], NEG)
    nc.vector.max(v[:, 16:24], s3[:, :w])
    nc.vector.max_index(ix[:, 16:24], v[:, 16:24], s3[:, :w])
    return v, ix


F16 = mybir.dt.float16


def _build_k1():
    nc = bacc.Bacc(None, target_bir_lowering=False)
    XT = nc.dram_tensor("XT", [PD, B], F32, kind="ExternalInput")
    PT = nc.dram_tensor("PT", [PD, NU], F32, kind="ExternalInput")
    XO = nc.dram_tensor("XO", [PD, 256], F32, kind="ExternalInput")
    RSH = nc.dram_tensor("RSH", [NU, SLICE], F16, kind="ExternalInput")
    RSL = nc.dram_tensor("RSL", [NU, SLICE], F16, kind="ExternalInput")
    WSP = nc.dram_tensor("WSP", [PD, LAT], F32, kind="ExternalInput")
    WMP = nc.dram_tensor("WMP", [PD, LAT], F32, kind="ExternalInput")
    WSD = nc.dram_tensor("WSD", [LAT, NT], F32, kind="ExternalInput")
    WMD = nc.dram_tensor("WMD", [LAT, NM], F32, kind="ExternalInput")
    CB = nc.dram_tensor("CB", [128, 2], F32, kind="ExternalInput")

    SIMV = nc.dram_tensor("SIMV", [B, 2 * NCAND], F32, kind="ExternalOutput")
    SIMI = nc.dram_tensor("SIMI", [B, 2 * NCAND], F32, kind="ExternalOutput")
    TPOS = nc.dram_tensor("TPOS", [256, NCAND], U32, kind="ExternalOutput")
    MPOS = nc.dram_tensor("MPOS", [256, 2 * NCAND], U32, kind="ExternalOutput")
    MVAL = nc.dram_tensor("MVAL", [256, 2 * NCAND], F32, kind="ExternalOutput")

    PRH = nc.dram_tensor("PRH", [NU, B], F16)   # internal: prior^T hi
    PRL = nc.dram_tensor("PRL", [NU, B], F16)   # internal: prior^T lo

    with tile.TileContext(nc) as tc:
        with (
            tc.tile_pool(name="cw", bufs=1) as cw,
            tc.tile_pool(name="tk", bufs=1) as tk,
            tc.tile_pool(name="ps", bufs=4, space="PSUM") as ps,
            tc.tile_pool(name="sbw", bufs=2) as sbw,
        ):
            cb = cw.tile([128, 2], F32, name="cb")
            nc.sync.dma_start(cb[:], CB[:])

            # Prefetch R half 0 up front; half 1 is later loaded into the SAME tiles
            # (range-level WAR lets each uc chunk reload as soon as its last half-0
            # read retires). Issued on the gpsimd SWDGE queue so the 20MB stream does
            # not clog the sync HWDGE queue that feeds stage A/C inputs and PRT writes.
            rsh0 = cw.tile([128, 16 * HALF], F16, tag="RH0", name="rsh0")
            rsl0 = cw.tile([128, 16 * HALF], F16, tag="RL0", name="rsl0")
            for uc in range(16):
                nc.gpsimd.dma_start(rsh0[:, uc * HALF:(uc + 1) * HALF],
                                    RSH[uc * 128:(uc + 1) * 128, 0:HALF])
                nc.gpsimd.dma_start(rsl0[:, uc * HALF:(uc + 1) * HALF],
                                    RSL[uc * 128:(uc + 1) * 128, 0:HALF])

            # ---------- Stage A: PRT[u, b] = sum_p PT[p,u] * XT[p,b] ----------
            with tc.tile_pool(name="stA", bufs=1) as sa, tc.tile_pool(name="stAw", bufs=2) as saw:
                xt = sa.tile([PD, B], F32, name="xt")
                nc.sync.dma_start(xt[:], XT[:])
                pt = sa.tile([PD, NU], F32, name="pt")
                nc.sync.dma_start(pt[:], PT[:])
                for uc in range(16):
                    for bt in range(4):
                        p = ps.tile([128, 512], F32, tag="p", name="pA")
                        nc.tensor.matmul(p[:], pt[:, uc * 128:(uc + 1) * 128],
                                         xt[:, bt * 512:(bt + 1) * 512], start=True, stop=True)
                        th = saw.tile([128, 512], F16, tag="ah", name="ah")
                        nc.scalar.copy(th[:], p[:])                       # hi = fp16(prior)
                        tl32 = saw.tile([128, 512], F32, tag="al32", name="al32")
                        nc.vector.tensor_tensor(tl32[:], p[:], th[:], ALU.subtract)
                        tl = saw.tile([128, 512], F16, tag="al", name="al")
                        nc.vector.tensor_copy(tl[:], tl32[:])             # lo = fp16(prior - hi)
                        nc.sync.dma_start(PRH[uc * 128:(uc + 1) * 128, bt * 512:(bt + 1) * 512], th[:])
                        nc.sync.dma_start(PRL[uc * 128:(uc + 1) * 128, bt * 512:(bt + 1) * 512], tl[:])

            # ---------- Stage C: own-row top/mid top-24 ----------
            with tc.tile_pool(name="stC", bufs=1) as sc, tc.tile_pool(name="stCw", bufs=2) as scw:
                xo = sc.tile([PD, 256], F32, name="xo")
                nc.sync.dma_start(xo[:], XO[:])
                wsp = sc.tile([PD, LAT], F32, name="wsp")
                nc.sync.dma_start(wsp[:], WSP[:])
                wmp = sc.tile([PD, LAT], F32, name="wmp")
                nc.sync.dma_start(wmp[:], WMP[:])
                wsd = sc.tile([LAT, NT], F32, name="wsd")
                nc.sync.dma_start(wsd[:], WSD[:])
                wmd = sc.tile([LAT, NM], F32, name="wmd")
                nc.sync.dma_start(wmd[:], WMD[:])

                ph1 = ps.tile([128, 512], F32, tag="p", name="ph1")
                nc.tensor.matmul(ph1[:, :256], wsp[:], xo[:], start=True, stop=True)
                h1t = sc.tile([LAT, 256], F32, name="h1t")
                nc.scalar.copy(h1t[:], ph1[:, :256])
                ph2 = ps.tile([128, 512], F32, tag="p", name="ph2")
                nc.tensor.matmul(ph2[:, :256], wmp[:], xo[:], start=True, stop=True)
                h1m = sc.tile([LAT, 256], F32, name="h1m")
                nc.scalar.copy(h1m[:], ph2[:, :256])

                for rb in range(2):
                    tsub = scw.tile([128, NT], F32, tag="sub", name="tsub")
                    for nt in range(4):
                        w = min(512, NT - nt * 512)
                        p = ps.tile([128, 512], F32, tag="p", name="pC")
                        nc.tensor.matmul(p[:, :w], h1t[:, rb * 128:(rb + 1) * 128],
                                         wsd[:, nt * 512: nt * 512 + w], start=True, stop=True)
                        nc.scalar.copy(tsub[:, nt * 512: nt * 512 + w], p[:, :w])
                    _, ixt = _topk24(nc, tk, tsub, NT)
                    nc.sync.dma_start(TPOS[rb * 128:(rb + 1) * 128, :], ixt[:])

                    for mh in range(2):
                        msub = scw.tile([128, MH], F32, tag="sub", name="msub")
                        for nt in range(5):
                            w = min(512, MH - nt * 512)
                            p = ps.tile([128, 512], F32, tag="p", name="pM")
                            nc.tensor.matmul(p[:, :w], h1m[:, rb * 128:(rb + 1) * 128],
                                             wmd[:, mh * MH + nt * 512: mh * MH + nt * 512 + w],
                                             start=True, stop=True)
                            nc.scalar.copy(msub[:, nt * 512: nt * 512 + w], p[:, :w])
                        vm, ixm = _topk24(nc, tk, msub, MH)
                        nc.sync.dma_start(MPOS[rb * 128:(rb + 1) * 128, mh * NCAND:(mh + 1) * NCAND], ixm[:])
                        nc.sync.dma_start(MVAL[rb * 128:(rb + 1) * 128, mh * NCAND:(mh + 1) * NCAND], vm[:])

            # ---------- Stage B: sim scoring (fp16 hi/lo split, 3 matmuls/chunk) + top-24 ----------
            def sim_half(h, rsh, rsl):
                for bc in range(16):
                    pbh = sbw.tile([128, 16 * 128], F16, tag="pbh", name="pbh")
                    nc.sync.dma_start(pbh[:], bass.AP(PRH, bc * 128, [[B, 128], [128 * B, 16], [1, 128]]))
                    pbl = sbw.tile([128, 16 * 128], F16, tag="pbl", name="pbl")
                    nc.sync.dma_start(pbl[:], bass.AP(PRL, bc * 128, [[B, 128], [128 * B, 16], [1, 128]]))
                    scores = sbw.tile([128, HALF], F32, tag="scores", name="scores")
                    for nt, w in ((0, 512), (512, 512), (1024, 226)):
                        p = ps.tile([128, 512], F32, tag="p", name="pB")
                        for uc in range(16):
                            first = uc == 0
                            last = uc == 15
                            nc.tensor.matmul(p[:, :w], pbh[:, uc * 128:(uc + 1) * 128],
                                             rsh[:, uc * HALF + nt: uc * HALF + nt + w],
                                             start=first, stop=False)
                            nc.tensor.matmul(p[:, :w], pbh[:, uc * 128:(uc + 1) * 128],
                                             rsl[:, uc * HALF + nt: uc * HALF + nt + w],
                                             start=False, stop=False)
                            nc.tensor.matmul(p[:, :w], pbl[:, uc * 128:(uc + 1) * 128],
                                             rsh[:, uc * HALF + nt: uc * HALF + nt + w],
                                             start=False, stop=last)
                        nc.scalar.copy(scores[:, nt:nt + w], p[:, :w])
                    v24, ix24 = _topk24(nc, tk, scores, HALF)
                    idsf = tk.tile([128, NCAND], F32, tag="idsf", name="idsf")
                    nc.vector.tensor_copy(idsf[:], ix24[:])
                    nc.vector.tensor_scalar_add(idsf[:], idsf[:], cb[:, h:h + 1])
                    nc.sync.dma_start(SIMV[bc * 128:(bc + 1) * 128, h * NCAND:(h + 1) * NCAND], v24[:])
                    nc.sync.dma_start(SIMI[bc * 128:(bc + 1) * 128, h * NCAND:(h + 1) * NCAND], idsf[:])

            sim_half(0, rsh0, rsl0)
            for uc in range(16):
                nc.gpsimd.dma_start(rsh0[:, uc * HALF:(uc + 1) * HALF],
                                    RSH[uc * 128:(uc + 1) * 128, HALF:2 * HALF])
                nc.gpsimd.dma_start(rsl0[:, uc * HALF:(uc + 1) * HALF],
                                    RSL[uc * 128:(uc + 1) * 128, HALF:2 * HALF])
            sim_half(1, rsh0, rsl0)
    nc.compile()
    return nc


# ----------------------------------------------------------------------------
# Kernel 2: exact fusion scan (see module docstring of the reference _fuse_one)
# ----------------------------------------------------------------------------
def _build_k2():
    nc = bacc.Bacc(None, target_bir_lowering=False)
    LT = nc.dram_tensor("LT", [256, K], F32, kind="ExternalInput")
    LM = nc.dram_tensor("LM", [256, K], F32, kind="ExternalInput")
    LS = nc.dram_tensor("LS", [256, K], F32, kind="ExternalInput")
    LOGP = nc.dram_tensor("LOGP", [256, 3], F32, kind="ExternalInput")
    GG = nc.dram_tensor("GG", [256, 60], F32, kind="ExternalInput")
    CON = nc.dram_tensor("CON", [128, 200], F32, kind="ExternalInput")
    OUT = nc.dram_tensor("OUT", [256, K], F32, kind="ExternalOutput")

    def v(tl, dims, off=0):
        base = tl[:]
        return bass.AP(base.tensor, base.offset + off, [list(base.ap[0])] + [list(d) for d in dims])

    with tile.TileContext(nc) as tc:
        with tc.tile_pool(name="p", bufs=1) as P:
            def T(w, name):
                return P.tile([128, w], F32, tag=name, name=name)

            t = T(40, "t"); m = T(40, "m"); s = T(40, "s")
            for blk in range(2):
                nc.sync.dma_start(t[:, blk * 20:(blk + 1) * 20], LT[blk * 128:(blk + 1) * 128, :])
                nc.sync.dma_start(m[:, blk * 20:(blk + 1) * 20], LM[blk * 128:(blk + 1) * 128, :])
                nc.sync.dma_start(s[:, blk * 20:(blk + 1) * 20], LS[blk * 128:(blk + 1) * 128, :])
            lp = T(6, "lp")
            for blk in range(2):
                nc.sync.dma_start(lp[:, blk * 3:(blk + 1) * 3], LOGP[blk * 128:(blk + 1) * 128, :])
            g = T(120, "g")
            for blk in range(2):
                nc.sync.dma_start(g[:, blk * 60:(blk + 1) * 60], GG[blk * 128:(blk + 1) * 128, :])
            con = T(200, "con")
            nc.sync.dma_start(con[:], CON[:])

            eq800 = T(800, "eq800")

            def isin(out, a, b, bmask=None):
                eq_v = v(eq800, [[400, 2], [20, 20], [1, 20]])
                a_v = v(a, [[20, 2], [1, 20], [0, 20]])
                b_v = v(b, [[20, 2], [0, 20], [1, 20]])
                nc.vector.tensor_tensor(eq_v, a_v, b_v, ALU.is_equal)
                if bmask is not None:
                    bm_v = v(bmask, [[20, 2], [0, 20], [1, 20]])
                    nc.vector.tensor_tensor(eq_v, eq_v, bm_v, ALU.mult)
                nc.vector.tensor_reduce(v(out, [[20, 2], [1, 20]]),
                                        v(eq800, [[400, 2], [20, 20], [1, 20]]), AX.X, ALU.max)

            def tnot(out, a):
                nc.vector.tensor_scalar(out[:], a[:], -1.0, 1.0, ALU.mult, ALU.add)

            def tmul(out, a, b):
                nc.vector.tensor_tensor(out[:], a[:], b[:], ALU.mult)

            mtm = T(40, "mtm"); mts = T(40, "mts"); mms = T(40, "mms")
            isin(mtm, t, m); isin(mts, t, s); isin(mms, m, s)
            mc = T(40, "mc")
            tmul(mc, mtm, mts); tmul(mc, mc, mms)
            icm = T(40, "icm")
            isin(icm, m, t, bmask=mc)
            ntc = T(40, "ntc"); tnot(ntc, mc)
            mtm2 = T(40, "mtm2"); tmul(mtm2, mtm, ntc)
            mts2 = T(40, "mts2"); tmul(mts2, mts, ntc)
            nicm = T(40, "nicm"); tnot(nicm, icm)
            mms2 = T(40, "mms2"); tmul(mms2, mms, nicm)
            tmp = T(40, "tmpa"); tmp2 = T(40, "tmpb")
            tpm = T(40, "tpm")
            tnot(tmp, mtm2); tnot(tmp2, mts2)
            tmul(tpm, ntc, tmp); tmul(tpm, tpm, tmp2)
            mpm = T(40, "mpm")
            isin(tmp, m, t, bmask=mtm2)
            tnot(tmp, tmp); tnot(tmp2, mms2)
            tmul(mpm, nicm, tmp); tmul(mpm, mpm, tmp2)
            spm = T(40, "spm")
            isin(tmp, s, t, bmask=mc); tnot(tmp, tmp)
            isin(tmp2, s, t, bmask=mts2); tnot(tmp2, tmp2)
            tmul(spm, tmp, tmp2)
            isin(tmp, s, m, bmask=mms2); tnot(tmp, tmp)
            tmul(spm, spm, tmp)

            # ---- det list ----
            dv = T(160, "dv"); dm = T(160, "dm")
            for j, srcm in enumerate((t, t, t, m)):
                nc.vector.tensor_copy(v(dv, [[80, 2], [1, 20]], j * 20), v(srcm, [[20, 2], [1, 20]]))
            for j, srcm in enumerate((mc, mtm2, mts2, mms2)):
                nc.vector.tensor_copy(v(dm, [[80, 2], [1, 20]], j * 20), v(srcm, [[20, 2], [1, 20]]))
            zz = T(80, "zz"); nc.vector.memset(zz[:], 0.0)
            cs = T(160, "cs")
            for blk in range(2):
                nc.vector.tensor_tensor_scan(cs[:, blk * 80:(blk + 1) * 80],
                                             dm[:, blk * 80:(blk + 1) * 80], zz[:],
                                             0.0, ALU.add, ALU.add)
            nd = T(2, "nd")
            nc.vector.tensor_copy(v(nd, [[1, 2]]), v(cs, [[80, 2]], 79))
            pos = T(160, "pos"); csm1 = T(160, "csm1")
            nc.vector.tensor_tensor(pos[:], v(con, [[0, 2], [1, 80]], 20), v(cs, [[80, 2], [1, 80]]), ALU.subtract)
            nc.vector.tensor_tensor(pos[:], pos[:], v(nd, [[1, 2], [0, 80]]), ALU.add)
            nc.vector.tensor_scalar(csm1[:], cs[:], -1.0, None, ALU.add)
            nc.vector.tensor_tensor(csm1[:], csm1[:], pos[:], ALU.subtract)
            nc.vector.tensor_tensor(csm1[:], csm1[:], dm[:], ALU.mult)
            nc.vector.tensor_tensor(pos[:], pos[:], csm1[:], ALU.add)
            E = T(3200, "E")
            E_v = v(E, [[1600, 2], [80, 20], [1, 80]])
            nc.vector.tensor_tensor(E_v, v(con, [[0, 2], [1, 20], [0, 80]], 0),
                                    v(pos, [[80, 2], [0, 20], [1, 80]]), ALU.is_equal)
            nc.vector.tensor_tensor(E_v, E_v, v(dv, [[80, 2], [0, 20], [1, 80]]), ALU.mult)
            det20 = T(40, "det20")
            nc.vector.tensor_reduce(v(det20, [[20, 2], [1, 20]]),
                                    v(E, [[1600, 2], [80, 20], [1, 80]]), AX.X, ALU.add)

            # ---- pools ----
            pv = T(120, "pv"); pm = T(120, "pm")
            for li, srcm in enumerate((t, m, s)):
                nc.vector.tensor_copy(v(pv, [[60, 2], [1, 20]], li * 20), v(srcm, [[20, 2], [1, 20]]))
            for li, srcm in enumerate((tpm, mpm, spm)):
                nc.vector.tensor_copy(v(pm, [[60, 2], [1, 20]], li * 20), v(srcm, [[20, 2], [1, 20]]))
            csp = T(120, "csp")
            for bl in range(6):
                nc.vector.tensor_tensor_scan(csp[:, bl * 20:(bl + 1) * 20],
                                             pm[:, bl * 20:(bl + 1) * 20], zz[:, :20],
                                             0.0, ALU.add, ALU.add)
            cnt = T(6, "cnt")
            nc.vector.tensor_copy(v(cnt, [[1, 6]]), v(csp, [[20, 6]], 19))
            pp = T(120, "pp"); cspm1 = T(120, "cspm1")
            nc.vector.tensor_tensor(pp[:], v(con, [[0, 6], [1, 20]], 0), v(csp, [[20, 6], [1, 20]]), ALU.subtract)
            nc.vector.tensor_tensor(pp[:], pp[:], v(cnt, [[1, 6], [0, 20]]), ALU.add)
            nc.vector.tensor_scalar(cspm1[:], csp[:], -1.0, None, ALU.add)
            nc.vector.tensor_tensor(cspm1[:], cspm1[:], pp[:], ALU.subtract)
            nc.vector.tensor_tensor(cspm1[:], cspm1[:], pm[:], ALU.mult)
            nc.vector.tensor_tensor(pp[:], pp[:], cspm1[:], ALU.add)
            E2 = T(2400, "E2")
            E2_v = v(E2, [[400, 6], [20, 20], [1, 20]])
            nc.vector.tensor_tensor(E2_v, v(con, [[0, 6], [1, 20], [0, 20]], 0),
                                    v(pp, [[20, 6], [0, 20], [1, 20]]), ALU.is_equal)
            nc.vector.tensor_tensor(E2_v, E2_v, v(pv, [[20, 6], [0, 20], [1, 20]]), ALU.mult)
            pc = T(120, "pc")
            nc.vector.tensor_reduce(v(pc, [[20, 6], [1, 20]]),
                                    v(E2, [[400, 6], [20, 20], [1, 20]]), AX.X, ALU.add)

            # ---- fusion scan ----
            sct = T(6, "sct")
            ptrA = T(6, "ptrA"); ptrB = T(6, "ptrB")
            nc.vector.memset(ptrA[:], 0.0)
            outb = T(40, "outb")
            av = T(6, "av"); sc2 = T(6, "sc2"); mx = T(2, "mx"); eqm = T(6, "eqm")
            i1 = T(2, "i1"); i2 = T(2, "i2"); idx = T(2, "idx"); anyav = T(2, "anyav")
            oh = T(6, "oh"); pm6 = T(6, "pm6"); psel = T(2, "psel"); tgt = T(2, "tgt")
            sel = T(120, "sel"); sp = T(120, "sp"); samp = T(2, "samp")
            used = T(2, "used"); d1 = T(2, "d1"); d2 = T(2, "d2"); val = T(2, "val")
            ndet = T(2, "ndet"); dp = T(6, "dp")
            for tstep in range(K):
                ptr, ptrn = (ptrA, ptrB) if tstep % 2 == 0 else (ptrB, ptrA)
                nc.vector.tensor_tensor(av[:], ptr[:], cnt[:], ALU.is_lt)
                nc.vector.tensor_tensor(sc2[:], lp[:], av[:], ALU.mult)
                nc.vector.tensor_scalar(sct[:], av[:], 3.0e38, -3.0e38, ALU.mult, ALU.add)
                nc.vector.tensor_tensor(sc2[:], sc2[:], sct[:], ALU.add)
                nc.vector.tensor_tensor(sc2[:], sc2[:], v(g, [[60, 2], [1, 3]], tstep * 3), ALU.add)
                nc.vector.tensor_reduce(v(mx, [[1, 2]]), v(sc2, [[3, 2], [1, 3]]), AX.X, ALU.max)
                nc.vector.tensor_tensor(eqm[:], sc2[:], v(mx, [[1, 2], [0, 3]]), ALU.is_equal)
                nc.vector.tensor_scalar(i1[:], v(eqm, [[3, 2]], 0), -1.0, 1.0, ALU.mult, ALU.add)
                nc.vector.tensor_scalar(i2[:], v(eqm, [[3, 2]], 1), -1.0, 2.0, ALU.mult, ALU.add)
                nc.vector.tensor_tensor(idx[:], i1[:], i2[:], ALU.mult)
                nc.vector.tensor_reduce(v(anyav, [[1, 2]]), v(av, [[3, 2], [1, 3]]), AX.X, ALU.max)
                nc.vector.tensor_tensor(idx[:], idx[:], anyav[:], ALU.mult)
                nc.vector.tensor_tensor(oh[:], v(con, [[1, 6]], 160), v(idx, [[1, 2], [0, 3]]), ALU.is_equal)
                nc.vector.tensor_tensor(pm6[:], ptr[:], oh[:], ALU.mult)
                nc.vector.tensor_reduce(v(psel, [[1, 2]]), v(pm6, [[3, 2], [1, 3]]), AX.X, ALU.add)
                nc.vector.tensor_scalar(psel[:], psel[:], float(K - 1), None, ALU.min)
                nc.vector.tensor_scalar(tgt[:], idx[:], 20.0, None, ALU.mult)
                nc.vector.tensor_tensor(tgt[:], tgt[:], psel[:], ALU.add)
                nc.vector.tensor_tensor(sel[:], v(con, [[0, 2], [1, 60]], 100),
                                        v(tgt, [[1, 2], [0, 60]]), ALU.is_equal)
                nc.vector.tensor_tensor(sp[:], sel[:], pc[:], ALU.mult)
                nc.vector.tensor_reduce(v(samp, [[1, 2]]), v(sp, [[60, 2], [1, 60]]), AX.X, ALU.add)
                nc.vector.tensor_scalar(used[:], nd[:], float(tstep), None, ALU.is_gt)
                nc.vector.tensor_tensor(d1[:], v(det20, [[20, 2]], tstep), samp[:], ALU.subtract)
                nc.vector.tensor_tensor(d2[:], used[:], d1[:], ALU.mult)
                nc.vector.tensor_tensor(val[:], samp[:], d2[:], ALU.add)
                nc.vector.tensor_copy(v(outb, [[20, 2]], tstep), val[:])
                nc.vector.tensor_scalar(ndet[:], used[:], -1.0, 1.0, ALU.mult, ALU.add)
                nc.vector.tensor_tensor(dp[:], oh[:], v(ndet, [[1, 2], [0, 3]]), ALU.mult)
                nc.vector.tensor_tensor(ptrn[:], ptr[:], dp[:], ALU.add)
            for blk in range(2):
                nc.sync.dma_start(OUT[blk * 128:(blk + 1) * 128, :], outb[:, blk * 20:(blk + 1) * 20])
    nc.compile()
    return nc


_K1 = None
_K2 = None


def _run_spmd(nc, in_maps, core_ids):
    """run_bass_kernel_spmd with retries: transient NRT device errors
    (e.g. NRT_EXEC_UNIT_UNRECOVERABLE right after another process released
    the cores) usually succeed on the next attempt."""
    last = None
    for _ in range(3):
        try:
            return run_bass_kernel_spmd(nc, in_maps, core_ids).results
        except Exception as e:   # noqa: BLE001
            last = e
            import time
            time.sleep(2.0)
    raise last


def _get_k1():
    global _K1
    if _K1 is None:
        _K1 = _build_k1()
    return _K1


def _get_k2():
    global _K2
    if _K2 is None:
        _K2 = _build_k2()
    return _K2


def _merge_desc(vals, ids, k):
    """Per-row top-k by value desc, ties broken by ascending id (jax.lax.top_k semantics)."""
    order = np.lexsort((ids, -vals.astype(np.float64)), axis=-1)[:, :k]
    return np.take_along_axis(ids, order, axis=1)


def kernel(**inputs):
    X = np.ascontiguousarray(np.asarray(inputs["X"], np.float32))
    W_sp = np.ascontiguousarray(np.asarray(inputs["W_sp"], np.float32))
    W_sd = np.ascontiguousarray(np.asarray(inputs["W_sd"], np.float32))
    W_mp = np.ascontiguousarray(np.asarray(inputs["W_mp"], np.float32))
    W_md = np.ascontiguousarray(np.asarray(inputs["W_md"], np.float32))
    W_mapper = np.ascontiguousarray(np.asarray(inputs["W_mapper"], np.float32))
    UR = np.ascontiguousarray(np.asarray(inputs["user_ratings"], np.float32))
    UP = np.ascontiguousarray(np.asarray(inputs["user_personalities"], np.float32))
    top_map = np.asarray(inputs["top_map"]).astype(np.int64)
    mid_map = np.asarray(inputs["mid_map"]).astype(np.int64)

    # ---------------- kernel 1 ----------------
    nc1 = _get_k1()
    XT = np.ascontiguousarray(X.T)
    PTp = np.zeros((PD, NU), np.float32)
    PTp[:, :UP.shape[0]] = UP.T
    in_maps1 = []
    URH = UR.astype(np.float16)
    URL = (UR - URH.astype(np.float32)).astype(np.float16)
    for c in range(NCORES):
        XO = np.ascontiguousarray(X[c * 256:(c + 1) * 256, :].T)
        RSh = np.zeros((NU, SLICE), np.float16)
        RSh[:UR.shape[0], :] = URH[:, c * SLICE:(c + 1) * SLICE]
        RSl = np.zeros((NU, SLICE), np.float16)
        RSl[:UR.shape[0], :] = URL[:, c * SLICE:(c + 1) * SLICE]
        cbase = np.empty((128, 2), np.float32)
        cbase[:, 0] = c * SLICE
        cbase[:, 1] = c * SLICE + HALF
        in_maps1.append({
            "XT": XT, "PT": PTp, "XO": XO, "RSH": RSh, "RSL": RSl,
            "WSP": W_sp, "WMP": W_mp, "WSD": W_sd, "WMD": W_md, "CB": cbase,
        })
    r1 = _run_spmd(nc1, in_maps1, list(range(NCORES)))

    # ---------------- host glue ----------------
    # top: positions already global within [0,2000): map through top_map
    tpos = np.concatenate([r1[c]["TPOS"] for c in range(NCORES)], axis=0).astype(np.int64)
    top20 = top_map[tpos[:, :K]].astype(np.float32)
    # mid: merge the two 2500-halves by value
    mpos = np.concatenate([r1[c]["MPOS"] for c in range(NCORES)], axis=0).astype(np.int64)
    mval = np.concatenate([r1[c]["MVAL"] for c in range(NCORES)], axis=0)
    mpos[:, NCAND:] += MH
    mid20 = mid_map[_merge_desc(mval, mpos, K)].astype(np.float32)
    # sim: merge 8 cores x 48 candidates by value
    simv = np.concatenate([r1[c]["SIMV"] for c in range(NCORES)], axis=1)   # [2048, 384]
    simi = np.concatenate([r1[c]["SIMI"] for c in range(NCORES)], axis=1).astype(np.int64)
    sim20 = _merge_desc(simv, simi, K).astype(np.float32)
    # probs / logp (tiny fp32 matmul; same rounding class as the reference's)
    z = X @ W_mapper
    zm = z - z.max(axis=1, keepdims=True)
    e = np.exp(zm)
    probs = (e / e.sum(axis=1, keepdims=True)).astype(np.float32)
    logp = np.log(probs).astype(np.float32)
    G = _gumbel_G(B, K, 42)

    # ---------------- kernel 2 ----------------
    nc2 = _get_k2()
    con = np.zeros((128, 200), np.float32)
    con[:, 0:20] = np.arange(20)
    con[:, 20:100] = np.arange(80)
    con[:, 100:160] = np.arange(60)
    con[:, 160:166] = np.array([0, 1, 2, 0, 1, 2])
    in_maps2 = []
    for c in range(NCORES):
        r = slice(c * 256, (c + 1) * 256)
        in_maps2.append({
            "LT": np.ascontiguousarray(top20[r]),
            "LM": np.ascontiguousarray(mid20[r]),
            "LS": np.ascontiguousarray(sim20[r]),
            "LOGP": np.ascontiguousarray(logp[r]),
            "GG": np.ascontiguousarray(G[r].reshape(256, 60)),
            "CON": con,
        })
    r2 = _run_spmd(nc2, in_maps2, list(range(NCORES)))
    out = np.concatenate([r2[c]["OUT"] for c in range(NCORES)], axis=0)
    return out.astype(np.float32)


# revision 18
# speedup vs baseline: 1.0458x; 1.0458x over previous
"""Trainium2 Bass kernel for nn_EnsembleModel_61718680044080 (nms_detection).

kernel(**inputs) -> [2048, 20] float32 (fused rec lists).

Plan (8 NeuronCores, SPMD):
  Kernel 1 (heavy):
    - prior^T = user_personalities @ X^T on PE (fp32), staged via internal DRAM.
    - sim scores = prior @ user_ratings, item-sharded: each core scores its 2500-item
      slice for ALL 2048 rows (fp32 PE matmuls), then per-row top-24 of each 1250-wide
      half on the vector engine (max8/max_index/match_replace rounds) -> candidate
      values + global item ids.
    - top/mid decoder paths, batch-sharded: each core computes top_sub [256,2000] and
      mid_sub [256,5000] for its own 256 rows and emits per-row top-24 positions
      (mid in two 2500-wide halves with values, merged on host).
  Host glue (numpy only): merge candidate lists by value, map positions through
    top_map/mid_map, softmax logits, and the Gumbel noise consumed by
    jax.random.categorical in the reference (replicated bit-exactly in numpy:
    threefry2x32 splits + XLA Philox4x32-10 bit stream, rbg PRNG impl).
  Kernel 2 (fusion): batch-sharded exact replication of the reference's greedy
    set-intersection fusion + 20-step sampling scan, fully vectorized on the DVE.
"""
import numpy as np
import concourse.bass as bass
import concourse.bacc as bacc
import concourse.mybir as mybir
from concourse import tile
from concourse.bass_utils import run_bass_kernel_spmd

F32 = mybir.dt.float32
U32 = mybir.dt.uint32
ALU = mybir.AluOpType
AX = mybir.AxisListType

B = 2048          # batch
PD = 64           # p_dim
LAT = 128         # latent
NU = 2048         # users (padded 2000 -> 2048)
NT = 2000         # n_top
NM = 5000         # n_mid
MH = 2500         # mid half width
SLICE = 2500      # sim items per core
HALF = 1250       # sim half width
NCAND = 24
NEG = -3.0e38
K = 20
NCORES = 8

# ----------------------------------------------------------------------------
# numpy replication of this environment's jax.random (rbg impl):
# threefry2x32 for key/split, XLA Philox4x32-10 for bits.
# ----------------------------------------------------------------------------
_ROT = ((13, 15, 26, 6), (17, 29, 16, 24))


def _threefry2x32(k0, k1, x0, x1):
    k0 = np.asarray(k0, np.uint32); k1 = np.asarray(k1, np.uint32)
    x0 = np.asarray(x0, np.uint32).copy(); x1 = np.asarray(x1, np.uint32).copy()
    with np.errstate(over="ignore"):
        ks2 = k0 ^ k1 ^ np.uint32(0x1BD11BDA)
        ks = (k0, k1, ks2)
        x0 = x0 + ks[0]; x1 = x1 + ks[1]
        for i in range(5):
            for r in _ROT[i % 2]:
                x0 = x0 + x1
                x1 = ((x1 << np.uint32(r)) | (x1 >> np.uint32(32 - r))) ^ x0
            x0 = x0 + ks[(i + 1) % 3]
            x1 = x1 + ks[(i + 2) % 3] + np.uint32(i + 1)
    return x0, x1


_M0 = np.uint64(0xD2511F53); _M1 = np.uint64(0xCD9E8D57)
_W0 = np.uint32(0x9E3779B9); _W1 = np.uint32(0xBB67AE85)
_MASK64 = np.uint64(0xFFFFFFFF)


def _philox_block(c0, c1, c2, c3, k0, k1):
    c = [np.asarray(c0, np.uint32).copy(), np.asarray(c1, np.uint32).copy(),
         np.asarray(c2, np.uint32).copy(), np.asarray(c3, np.uint32).copy()]
    k0 = np.asarray(k0, np.uint32).copy(); k1 = np.asarray(k1, np.uint32).copy()
    with np.errstate(over="ignore"):
        for _ in range(10):
            p0 = _M0 * c[0].astype(np.uint64)
            p1 = _M1 * c[2].astype(np.uint64)
            hi0 = (p0 >> np.uint64(32)).astype(np.uint32); lo0 = (p0 & _MASK64).astype(np.uint32)
            hi1 = (p1 >> np.uint64(32)).astype(np.uint32); lo1 = (p1 & _MASK64).astype(np.uint32)
            c = [hi1 ^ c[1] ^ k0, lo1, hi0 ^ c[3] ^ k1, lo0]
            k0 = k0 + _W0; k1 = k1 + _W1
    return c


def _philox_stream(k0, k1, n_u32):
    nblk = (n_u32 + 3) // 4
    c64 = (np.uint64(k1) << np.uint64(32)) | np.uint64(k0)
    with np.errstate(over="ignore"):
        cs = c64 + np.arange(nblk, dtype=np.uint64)
    clo = (cs & _MASK64).astype(np.uint32)
    chi = (cs >> np.uint64(32)).astype(np.uint32)
    w = _philox_block(clo, chi, np.full(nblk, k0, np.uint32), np.full(nblk, k1, np.uint32),
                      np.full(nblk, k0, np.uint32), np.full(nblk, k1, np.uint32))
    out = np.empty((nblk, 4), np.uint32)
    for j in range(4):
        out[:, j] = w[j]
    return out.reshape(-1)[:n_u32]


def _gumbel_G(Bn=B, Kn=K, seed=42):
    """G[b,t,c]: gumbel noise consumed by the reference's vmapped categorical scan.
    Under vmap, the rbg impl draws each step's whole [B,3] block from ROW 0's sub key."""
    kk = np.array([(seed >> 32) & 0xFFFFFFFF, seed & 0xFFFFFFFF], np.uint32)
    y0, y1 = _threefry2x32(kk[0], kk[1], np.zeros(Bn, np.uint32), np.arange(Bn, dtype=np.uint32))
    a0, b0 = y0[0], y1[0]               # row 0's key words
    G = np.empty((Bn, Kn, 3), np.float32)
    tiny = np.float32(np.finfo(np.float32).tiny)
    onef = np.float32(1.0)
    for t in range(Kn):
        n0, n1 = _threefry2x32(a0, b0, 0, 0)
        s0, s1 = _threefry2x32(a0, b0, 0, 1)
        bits = _philox_stream(np.uint32(s0), np.uint32(s1), Bn * 3)
        u = ((bits >> np.uint32(9)) | np.uint32(0x3F800000)).view(np.float32) - onef
        u = u * (onef - tiny) + tiny
        np.maximum(u, tiny, out=u)
        G[:, t, :] = (-np.log(-np.log(u))).reshape(Bn, 3)
        a0, b0 = n0, n1
    return G


# ----------------------------------------------------------------------------
# Kernel 1 (heavy): prior^T, sim scoring + top-24, top/mid top-24 positions
# ----------------------------------------------------------------------------
def _topk24(nc, pool, s, w):
    v = pool.tile([128, NCAND], F32, tag="tk_v", name="tk_v")
    ix = pool.tile([128, NCAND], U32, tag="tk_i", name="tk_i")
    s2 = pool.tile([128, 2560], F32, tag="tk_s2", name="tk_s2")
    s3 = pool.tile([128, 2560], F32, tag="tk_s3", name="tk_s3")
    nc.vector.max(v[:, 0:8], s[:])
    nc.vector.max_index(ix[:, 0:8], v[:, 0:8], s[:])
    nc.vector.match_replace(s2[:, :w], v[:, 0:8], s[:], NEG)
    nc.vector.max(v[:, 8:16], s2[:, :w])
    nc.vector.max_index(ix[:, 8:16], v[:, 8:16], s2[:, :w])
    nc.vector.match_replace(s3[:, :w], v[:, 8:16], s2[:, :w], NEG)
    nc.vector.max(v[:, 16:24], s3[:, :w])
    nc.vector.max_index(ix[:, 16:24], v[:, 16:24], s3[:, :w])
    return v, ix


F16 = mybir.dt.float16


def _build_k1():
    nc = bacc.Bacc(None, target_bir_lowering=False)
    XT = nc.dram_tensor("XT", [PD, B], F32, kind="ExternalInput")
    PT = nc.dram_tensor("PT", [PD, NU], F32, kind="ExternalInput")
    XO = nc.dram_tensor("XO", [PD, 256], F32, kind="ExternalInput")
    RSH = nc.dram_tensor("RSH", [NU, SLICE], F16, kind="ExternalInput")
    RSL = nc.dram_tensor("RSL", [NU, SLICE], F16, kind="ExternalInput")
    WSP = nc.dram_tensor("WSP", [PD, LAT], F32, kind="ExternalInput")
    WMP = nc.dram_tensor("WMP", [PD, LAT], F32, kind="ExternalInput")
    WSD = nc.dram_tensor("WSD", [LAT, NT], F32, kind="ExternalInput")
    WMD = nc.dram_tensor("WMD", [LAT, NM], F32, kind="ExternalInput")
    CB = nc.dram_tensor("CB", [128, 2], F32, kind="ExternalInput")

    SIMV = nc.dram_tensor("SIMV", [B, 2 * NCAND], F32, kind="ExternalOutput")
    SIMI = nc.dram_tensor("SIMI", [B, 2 * NCAND], F32, kind="ExternalOutput")
    TPOS = nc.dram_tensor("TPOS", [256, NCAND], U32, kind="ExternalOutput")
    MPOS = nc.dram_tensor("MPOS", [256, 2 * NCAND], U32, kind="ExternalOutput")
    MVAL = nc.dram_tensor("MVAL", [256, 2 * NCAND], F32, kind="ExternalOutput")

    PRH = nc.dram_tensor("PRH", [NU, B], F16)   # internal: prior^T hi
    PRL = nc.dram_tensor("PRL", [NU, B], F16)   # internal: prior^T lo

    with tile.TileContext(nc) as tc:
        with (
            tc.tile_pool(name="cw", bufs=1) as cw,
            tc.tile_pool(name="tk", bufs=1) as tk,
            tc.tile_pool(name="ps", bufs=4, space="PSUM") as ps,
            tc.tile_pool(name="sbw", bufs=2) as sbw,
        ):
            cb = cw.tile([128, 2], F32, name="cb")
            nc.sync.dma_start(cb[:], CB[:])

            # Prefetch R half 0 up front; half 1 is later loaded into the SAME tiles
            # (range-level WAR lets each uc chunk reload as soon as its last half-0
            # read retires). Issued on the gpsimd SWDGE queue so the 20MB stream does
            # not clog the sync HWDGE queue that feeds stage A/C inputs and PRT writes.
            rshs = []
            rsls = []
            for uc in range(16):
                th = cw.tile([128, HALF], F16, tag=f"RH{uc}", name=f"rsh{uc}")
                tl = cw.tile([128, HALF], F16, tag=f"RL{uc}", name=f"rsl{uc}")
                nc.gpsimd.dma_start(th[:], RSH[uc * 128:(uc + 1) * 128, 0:HALF])
                nc.gpsimd.dma_start(tl[:], RSL[uc * 128:(uc + 1) * 128, 0:HALF])
                rshs.append(th)
                rsls.append(tl)

            # ---------- Stage A: PRT[u, b] = sum_p PT[p,u] * XT[p,b] ----------
            with tc.tile_pool(name="stA", bufs=1) as sa, tc.tile_pool(name="stAw", bufs=2) as saw:
                xt = sa.tile([PD, B], F32, name="xt")
                nc.sync.dma_start(xt[:], XT[:])
                pt = sa.tile([PD, NU], F32, name="pt")
                nc.sync.dma_start(pt[:], PT[:])
                for uc in range(16):
                    for bt in range(4):
                        p = ps.tile([128, 512], F32, tag="p", name="pA")
                        nc.tensor.matmul(p[:], pt[:, uc * 128:(uc + 1) * 128],
                                         xt[:, bt * 512:(bt + 1) * 512], start=True, stop=True)
                        th = saw.tile([128, 512], F16, tag="ah", name="ah")
                        nc.scalar.copy(th[:], p[:])                       # hi = fp16(prior)
                        tl32 = saw.tile([128, 512], F32, tag="al32", name="al32")
                        nc.vector.tensor_tensor(tl32[:], p[:], th[:], ALU.subtract)
                        tl = saw.tile([128, 512], F16, tag="al", name="al")
                        nc.vector.tensor_copy(tl[:], tl32[:])             # lo = fp16(prior - hi)
                        nc.sync.dma_start(PRH[uc * 128:(uc + 1) * 128, bt * 512:(bt + 1) * 512], th[:])
                        nc.sync.dma_start(PRL[uc * 128:(uc + 1) * 128, bt * 512:(bt + 1) * 512], tl[:])

            # ---------- Stage C: own-row top/mid top-24 ----------
            # Setup runs before stage B; the per-(rb,list) units are emitted
            # interleaved between early stage-B iterations (CPOINTS) so their
            # PSUM-copy/top-k latency hides under stage B's matmul stream.
            sccm = tc.tile_pool(name="stC", bufs=1)
            scp = sccm.__enter__()
            scwcm = tc.tile_pool(name="stCw", bufs=2)
            scw = scwcm.__enter__()
            xo = scp.tile([PD, 256], F32, name="xo")
            nc.sync.dma_start(xo[:], XO[:])
            wsp = scp.tile([PD, LAT], F32, name="wsp")
            nc.sync.dma_start(wsp[:], WSP[:])
            wmp = scp.tile([PD, LAT], F32, name="wmp")
            nc.sync.dma_start(wmp[:], WMP[:])
            wsd = scp.tile([LAT, NT], F32, name="wsd")
            nc.sync.dma_start(wsd[:], WSD[:])
            wmd = scp.tile([LAT, NM], F32, name="wmd")
            nc.sync.dma_start(wmd[:], WMD[:])

            ph1 = ps.tile([128, 512], F32, tag="p", name="ph1")
            nc.tensor.matmul(ph1[:, :256], wsp[:], xo[:], start=True, stop=True)
            h1t = scp.tile([LAT, 256], F32, name="h1t")
            nc.scalar.copy(h1t[:], ph1[:, :256])
            ph2 = ps.tile([128, 512], F32, tag="p", name="ph2")
            nc.tensor.matmul(ph2[:, :256], wmp[:], xo[:], start=True, stop=True)
            h1m = scp.tile([LAT, 256], F32, name="h1m")
            nc.scalar.copy(h1m[:], ph2[:, :256])

            def c_top(rb):
                tsub = scw.tile([128, NT], F32, tag="sub", name="tsub")
                for nt in range(4):
                    w = min(512, NT - nt * 512)
                    p = ps.tile([128, 512], F32, tag="p", name="pC")
                    nc.tensor.matmul(p[:, :w], h1t[:, rb * 128:(rb + 1) * 128],
                                     wsd[:, nt * 512: nt * 512 + w], start=True, stop=True)
                    nc.scalar.copy(tsub[:, nt * 512: nt * 512 + w], p[:, :w])
                _, ixt = _topk24(nc, tk, tsub, NT)
                nc.sync.dma_start(TPOS[rb * 128:(rb + 1) * 128, :], ixt[:])

            def c_mid(rb, mh):
                msub = scw.tile([128, MH], F32, tag="sub", name="msub")
                for nt in range(5):
                    w = min(512, MH - nt * 512)
                    p = ps.tile([128, 512], F32, tag="p", name="pM")
                    nc.tensor.matmul(p[:, :w], h1m[:, rb * 128:(rb + 1) * 128],
                                     wmd[:, mh * MH + nt * 512: mh * MH + nt * 512 + w],
                                     start=True, stop=True)
                    nc.scalar.copy(msub[:, nt * 512: nt * 512 + w], p[:, :w])
                vm, ixm = _topk24(nc, tk, msub, MH)
                nc.sync.dma_start(MPOS[rb * 128:(rb + 1) * 128, mh * NCAND:(mh + 1) * NCAND], ixm[:])
                nc.sync.dma_start(MVAL[rb * 128:(rb + 1) * 128, mh * NCAND:(mh + 1) * NCAND], vm[:])

            C_UNITS = [lambda: c_top(0), lambda: c_mid(0, 0), lambda: c_mid(0, 1),
                       lambda: c_top(1), lambda: c_mid(1, 0), lambda: c_mid(1, 1)]
            CPOINTS = {1: 0, 3: 1, 5: 2, 7: 3, 9: 4, 11: 5}

            def emit_c(i):
                C_UNITS[i]()

            # ---------- Stage B: sim scoring (fp16 hi/lo split, 3 matmuls/chunk) + top-24 ----------
            def sim_bc(h, bc, rsh, rsl):
                if True:
                    pbh = sbw.tile([128, 16 * 128], F16, tag="pbh", name="pbh")
                    nc.sync.dma_start(pbh[:], bass.AP(PRH, bc * 128, [[B, 128], [128 * B, 16], [1, 128]]))
                    pbl = sbw.tile([128, 16 * 128], F16, tag="pbl", name="pbl")
                    nc.sync.dma_start(pbl[:], bass.AP(PRL, bc * 128, [[B, 128], [128 * B, 16], [1, 128]]))
                    scores = sbw.tile([128, HALF], F32, tag="scores", name="scores")
                    for nt, w in ((0, 512), (512, 512), (1024, 226)):
                        p = ps.tile([128, 512], F32, tag="p", name="pB")
                        for uc in range(16):
                            first = uc == 0
                            last = uc == 15
                            nc.tensor.matmul(p[:, :w], pbh[:, uc * 128:(uc + 1) * 128],
                                             rsh[uc][:, nt: nt + w], start=first, stop=False)
                            nc.tensor.matmul(p[:, :w], pbh[:, uc * 128:(uc + 1) * 128],
                                             rsl[uc][:, nt: nt + w], start=False, stop=False)
                            nc.tensor.matmul(p[:, :w], pbl[:, uc * 128:(uc + 1) * 128],
                                             rsh[uc][:, nt: nt + w], start=False, stop=last)
                        nc.scalar.copy(scores[:, nt:nt + w], p[:, :w])
                    v24, ix24 = _topk24(nc, tk, scores, HALF)
                    idsf = tk.tile([128, NCAND], F32, tag="idsf", name="idsf")
                    nc.vector.tensor_copy(idsf[:], ix24[:])
                    nc.vector.tensor_scalar_add(idsf[:], idsf[:], cb[:, h:h + 1])
                    nc.sync.dma_start(SIMV[bc * 128:(bc + 1) * 128, h * NCAND:(h + 1) * NCAND], v24[:])
                    nc.sync.dma_start(SIMI[bc * 128:(bc + 1) * 128, h * NCAND:(h + 1) * NCAND], idsf[:])

            for bc in range(16):
                sim_bc(0, bc, rshs, rsls)
                if bc in CPOINTS:
                    emit_c(CPOINTS[bc])
            for uc in range(16):
                nc.gpsimd.dma_start(rshs[uc][:], RSH[uc * 128:(uc + 1) * 128, HALF:2 * HALF])
                nc.gpsimd.dma_start(rsls[uc][:], RSL[uc * 128:(uc + 1) * 128, HALF:2 * HALF])
            for bc in range(16):
                sim_bc(1, bc, rshs, rsls)
            scwcm.__exit__(None, None, None)
            sccm.__exit__(None, None, None)
    nc.compile()
    return nc


# ----------------------------------------------------------------------------
# Kernel 2: exact fusion scan (see module docstring of the reference _fuse_one)
# ----------------------------------------------------------------------------
def _build_k2():
    nc = bacc.Bacc(None, target_bir_lowering=False)
    LT = nc.dram_tensor("LT", [256, K], F32, kind="ExternalInput")
    LM = nc.dram_tensor("LM", [256, K], F32, kind="ExternalInput")
    LS = nc.dram_tensor("LS", [256, K], F32, kind="ExternalInput")
    LOGP = nc.dram_tensor("LOGP", [256, 3], F32, kind="ExternalInput")
    GG = nc.dram_tensor("GG", [256, 60], F32, kind="ExternalInput")
    CON = nc.dram_tensor("CON", [128, 200], F32, kind="ExternalInput")
    OUT = nc.dram_tensor("OUT", [256, K], F32, kind="ExternalOutput")

    def v(tl, dims, off=0):
        base = tl[:]
        return bass.AP(base.tensor, base.offset + off, [list(base.ap[0])] + [list(d) for d in dims])

    with tile.TileContext(nc) as tc:
        with tc.tile_pool(name="p", bufs=1) as P:
            def T(w, name):
                return P.tile([128, w], F32, tag=name, name=name)

            t = T(40, "t"); m = T(40, "m"); s = T(40, "s")
            for blk in range(2):
                nc.sync.dma_start(t[:, blk * 20:(blk + 1) * 20], LT[blk * 128:(blk + 1) * 128, :])
                nc.sync.dma_start(m[:, blk * 20:(blk + 1) * 20], LM[blk * 128:(blk + 1) * 128, :])
                nc.sync.dma_start(s[:, blk * 20:(blk + 1) * 20], LS[blk * 128:(blk + 1) * 128, :])
            lp = T(6, "lp")
            for blk in range(2):
                nc.sync.dma_start(lp[:, blk * 3:(blk + 1) * 3], LOGP[blk * 128:(blk + 1) * 128, :])
            g = T(120, "g")
            for blk in range(2):
                nc.sync.dma_start(g[:, blk * 60:(blk + 1) * 60], GG[blk * 128:(blk + 1) * 128, :])
            con = T(200, "con")
            nc.sync.dma_start(con[:], CON[:])

            eq800 = T(800, "eq800")

            def isin(out, a, b, bmask=None):
                eq_v = v(eq800, [[400, 2], [20, 20], [1, 20]])
                a_v = v(a, [[20, 2], [1, 20], [0, 20]])
                b_v = v(b, [[20, 2], [0, 20], [1, 20]])
                nc.vector.tensor_tensor(eq_v, a_v, b_v, ALU.is_equal)
                if bmask is not None:
                    bm_v = v(bmask, [[20, 2], [0, 20], [1, 20]])
                    nc.vector.tensor_tensor(eq_v, eq_v, bm_v, ALU.mult)
                nc.vector.tensor_reduce(v(out, [[20, 2], [1, 20]]),
                                        v(eq800, [[400, 2], [20, 20], [1, 20]]), AX.X, ALU.max)

            def tnot(out, a):
                nc.vector.tensor_scalar(out[:], a[:], -1.0, 1.0, ALU.mult, ALU.add)

            def tmul(out, a, b):
                nc.vector.tensor_tensor(out[:], a[:], b[:], ALU.mult)

            mtm = T(40, "mtm"); mts = T(40, "mts"); mms = T(40, "mms")
            isin(mtm, t, m); isin(mts, t, s); isin(mms, m, s)
            mc = T(40, "mc")
            tmul(mc, mtm, mts); tmul(mc, mc, mms)
            icm = T(40, "icm")
            isin(icm, m, t, bmask=mc)
            ntc = T(40, "ntc"); tnot(ntc, mc)
            mtm2 = T(40, "mtm2"); tmul(mtm2, mtm, ntc)
            mts2 = T(40, "mts2"); tmul(mts2, mts, ntc)
            nicm = T(40, "nicm"); tnot(nicm, icm)
            mms2 = T(40, "mms2"); tmul(mms2, mms, nicm)
            tmp = T(40, "tmpa"); tmp2 = T(40, "tmpb")
            tpm = T(40, "tpm")
            tnot(tmp, mtm2); tnot(tmp2, mts2)
            tmul(tpm, ntc, tmp); tmul(tpm, tpm, tmp2)
            mpm = T(40, "mpm")
            isin(tmp, m, t, bmask=mtm2)
            tnot(tmp, tmp); tnot(tmp2, mms2)
            tmul(mpm, nicm, tmp); tmul(mpm, mpm, tmp2)
            spm = T(40, "spm")
            isin(tmp, s, t, bmask=mc); tnot(tmp, tmp)
            isin(tmp2, s, t, bmask=mts2); tnot(tmp2, tmp2)
            tmul(spm, tmp, tmp2)
            isin(tmp, s, m, bmask=mms2); tnot(tmp, tmp)
            tmul(spm, spm, tmp)

            # ---- det list ----
            dv = T(160, "dv"); dm = T(160, "dm")
            for j, srcm in enumerate((t, t, t, m)):
                nc.vector.tensor_copy(v(dv, [[80, 2], [1, 20]], j * 20), v(srcm, [[20, 2], [1, 20]]))
            for j, srcm in enumerate((mc, mtm2, mts2, mms2)):
                nc.vector.tensor_copy(v(dm, [[80, 2], [1, 20]], j * 20), v(srcm, [[20, 2], [1, 20]]))
            zz = T(80, "zz"); nc.vector.memset(zz[:], 0.0)
            cs = T(160, "cs")
            for blk in range(2):
                nc.vector.tensor_tensor_scan(cs[:, blk * 80:(blk + 1) * 80],
                                             dm[:, blk * 80:(blk + 1) * 80], zz[:],
                                             0.0, ALU.add, ALU.add)
            nd = T(2, "nd")
            nc.vector.tensor_copy(v(nd, [[1, 2]]), v(cs, [[80, 2]], 79))
            pos = T(160, "pos"); csm1 = T(160, "csm1")
            nc.vector.tensor_tensor(pos[:], v(con, [[0, 2], [1, 80]], 20), v(cs, [[80, 2], [1, 80]]), ALU.subtract)
            nc.vector.tensor_tensor(pos[:], pos[:], v(nd, [[1, 2], [0, 80]]), ALU.add)
            nc.vector.tensor_scalar(csm1[:], cs[:], -1.0, None, ALU.add)
            nc.vector.tensor_tensor(csm1[:], csm1[:], pos[:], ALU.subtract)
            nc.vector.tensor_tensor(csm1[:], csm1[:], dm[:], ALU.mult)
            nc.vector.tensor_tensor(pos[:], pos[:], csm1[:], ALU.add)
            E = T(3200, "E")
            E_v = v(E, [[1600, 2], [80, 20], [1, 80]])
            nc.vector.tensor_tensor(E_v, v(con, [[0, 2], [1, 20], [0, 80]], 0),
                                    v(pos, [[80, 2], [0, 20], [1, 80]]), ALU.is_equal)
            nc.vector.tensor_tensor(E_v, E_v, v(dv, [[80, 2], [0, 20], [1, 80]]), ALU.mult)
            det20 = T(40, "det20")
            nc.vector.tensor_reduce(v(det20, [[20, 2], [1, 20]]),
                                    v(E, [[1600, 2], [80, 20], [1, 80]]), AX.X, ALU.add)

            # ---- pools ----
            pv = T(120, "pv"); pm = T(120, "pm")
            for li, srcm in enumerate((t, m, s)):
                nc.vector.tensor_copy(v(pv, [[60, 2], [1, 20]], li * 20), v(srcm, [[20, 2], [1, 20]]))
            for li, srcm in enumerate((tpm, mpm, spm)):
                nc.vector.tensor_copy(v(pm, [[60, 2], [1, 20]], li * 20), v(srcm, [[20, 2], [1, 20]]))
            csp = T(120, "csp")
            for bl in range(6):
                nc.vector.tensor_tensor_scan(csp[:, bl * 20:(bl + 1) * 20],
                                             pm[:, bl * 20:(bl + 1) * 20], zz[:, :20],
                                             0.0, ALU.add, ALU.add)
            cnt = T(6, "cnt")
            nc.vector.tensor_copy(v(cnt, [[1, 6]]), v(csp, [[20, 6]], 19))
            pp = T(120, "pp"); cspm1 = T(120, "cspm1")
            nc.vector.tensor_tensor(pp[:], v(con, [[0, 6], [1, 20]], 0), v(csp, [[20, 6], [1, 20]]), ALU.subtract)
            nc.vector.tensor_tensor(pp[:], pp[:], v(cnt, [[1, 6], [0, 20]]), ALU.add)
            nc.vector.tensor_scalar(cspm1[:], csp[:], -1.0, None, ALU.add)
            nc.vector.tensor_tensor(cspm1[:], cspm1[:], pp[:], ALU.subtract)
            nc.vector.tensor_tensor(cspm1[:], cspm1[:], pm[:], ALU.mult)
            nc.vector.tensor_tensor(pp[:], pp[:], cspm1[:], ALU.add)
            E2 = T(2400, "E2")
            E2_v = v(E2, [[400, 6], [20, 20], [1, 20]])
            nc.vector.tensor_tensor(E2_v, v(con, [[0, 6], [1, 20], [0, 20]], 0),
                                    v(pp, [[20, 6], [0, 20], [1, 20]]), ALU.is_equal)
            nc.vector.tensor_tensor(E2_v, E2_v, v(pv, [[20, 6], [0, 20], [1, 20]]), ALU.mult)
            pc = T(120, "pc")
            nc.vector.tensor_reduce(v(pc, [[20, 6], [1, 20]]),
                                    v(E2, [[400, 6], [20, 20], [1, 20]]), AX.X, ALU.add)

            # ---- fusion scan ----
            sct = T(6, "sct")
            ptrA = T(6, "ptrA"); ptrB = T(6, "ptrB")
            nc.vector.memset(ptrA[:], 0.0)
            outb = T(40, "outb")
            av = T(6, "av"); sc2 = T(6, "sc2"); mx = T(2, "mx"); eqm = T(6, "eqm")
            i1 = T(2, "i1"); i2 = T(2, "i2"); idx = T(2, "idx"); anyav = T(2, "anyav")
            oh = T(6, "oh"); pm6 = T(6, "pm6"); psel = T(2, "psel"); tgt = T(2, "tgt")
            sel = T(120, "sel"); sp = T(120, "sp"); samp = T(2, "samp")
            used = T(2, "used"); d1 = T(2, "d1"); d2 = T(2, "d2"); val = T(2, "val")
            ndet = T(2, "ndet"); dp = T(6, "dp")
            for tstep in range(K):
                ptr, ptrn = (ptrA, ptrB) if tstep % 2 == 0 else (ptrB, ptrA)
                nc.vector.tensor_tensor(av[:], ptr[:], cnt[:], ALU.is_lt)
                nc.vector.tensor_tensor(sc2[:], lp[:], av[:], ALU.mult)
                nc.vector.tensor_scalar(sct[:], av[:], 3.0e38, -3.0e38, ALU.mult, ALU.add)
                nc.vector.tensor_tensor(sc2[:], sc2[:], sct[:], ALU.add)
                nc.vector.tensor_tensor(sc2[:], sc2[:], v(g, [[60, 2], [1, 3]], tstep * 3), ALU.add)
                nc.vector.tensor_reduce(v(mx, [[1, 2]]), v(sc2, [[3, 2], [1, 3]]), AX.X, ALU.max)
                nc.vector.tensor_tensor(eqm[:], sc2[:], v(mx, [[1, 2], [0, 3]]), ALU.is_equal)
                nc.vector.tensor_scalar(i1[:], v(eqm, [[3, 2]], 0), -1.0, 1.0, ALU.mult, ALU.add)
                nc.vector.tensor_scalar(i2[:], v(eqm, [[3, 2]], 1), -1.0, 2.0, ALU.mult, ALU.add)
                nc.vector.tensor_tensor(idx[:], i1[:], i2[:], ALU.mult)
                nc.vector.tensor_reduce(v(anyav, [[1, 2]]), v(av, [[3, 2], [1, 3]]), AX.X, ALU.max)
                nc.vector.tensor_tensor(idx[:], idx[:], anyav[:], ALU.mult)
                nc.vector.tensor_tensor(oh[:], v(con, [[1, 6]], 160), v(idx, [[1, 2], [0, 3]]), ALU.is_equal)
                nc.vector.tensor_tensor(pm6[:], ptr[:], oh[:], ALU.mult)
                nc.vector.tensor_reduce(v(psel, [[1, 2]]), v(pm6, [[3, 2], [1, 3]]), AX.X, ALU.add)
                nc.vector.tensor_scalar(psel[:], psel[:], float(K - 1), None, ALU.min)
                nc.vector.tensor_scalar(tgt[:], idx[:], 20.0, None, ALU.mult)
                nc.vector.tensor_tensor(tgt[:], tgt[:], psel[:], ALU.add)
                nc.vector.tensor_tensor(sel[:], v(con, [[0, 2], [1, 60]], 100),
                                        v(tgt, [[1, 2], [0, 60]]), ALU.is_equal)
                nc.vector.tensor_tensor(sp[:], sel[:], pc[:], ALU.mult)
                nc.vector.tensor_reduce(v(samp, [[1, 2]]), v(sp, [[60, 2], [1, 60]]), AX.X, ALU.add)
                nc.vector.tensor_scalar(used[:], nd[:], float(tstep), None, ALU.is_gt)
                nc.vector.tensor_tensor(d1[:], v(det20, [[20, 2]], tstep), samp[:], ALU.subtract)
                nc.vector.tensor_tensor(d2[:], used[:], d1[:], ALU.mult)
                nc.vector.tensor_tensor(val[:], samp[:], d2[:], ALU.add)
                nc.vector.tensor_copy(v(outb, [[20, 2]], tstep), val[:])
                nc.vector.tensor_scalar(ndet[:], used[:], -1.0, 1.0, ALU.mult, ALU.add)
                nc.vector.tensor_tensor(dp[:], oh[:], v(ndet, [[1, 2], [0, 3]]), ALU.mult)
                nc.vector.tensor_tensor(ptrn[:], ptr[:], dp[:], ALU.add)
            for blk in range(2):
                nc.sync.dma_start(OUT[blk * 128:(blk + 1) * 128, :], outb[:, blk * 20:(blk + 1) * 20])
    nc.compile()
    return nc


_K1 = None
_K2 = None


def _run_spmd(nc, in_maps, core_ids):
    """run_bass_kernel_spmd with retries: transient NRT device errors
    (e.g. NRT_EXEC_UNIT_UNRECOVERABLE right after another process released
    the cores) usually succeed on the next attempt."""
    last = None
    for _ in range(3):
        try:
            return run_bass_kernel_spmd(nc, in_maps, core_ids).results
        except Exception as e:   # noqa: BLE001
            last = e
            import time
            time.sleep(2.0)
    raise last


def _get_k1():
    global _K1
    if _K1 is None:
        _K1 = _build_k1()
    return _K1


def _get_k2():
    global _K2
    if _K2 is None:
        _K2 = _build_k2()
    return _K2


def _merge_desc(vals, ids, k):
    """Per-row top-k by value desc, ties broken by ascending id (jax.lax.top_k semantics)."""
    order = np.lexsort((ids, -vals.astype(np.float64)), axis=-1)[:, :k]
    return np.take_along_axis(ids, order, axis=1)


def kernel(**inputs):
    X = np.ascontiguousarray(np.asarray(inputs["X"], np.float32))
    W_sp = np.ascontiguousarray(np.asarray(inputs["W_sp"], np.float32))
    W_sd = np.ascontiguousarray(np.asarray(inputs["W_sd"], np.float32))
    W_mp = np.ascontiguousarray(np.asarray(inputs["W_mp"], np.float32))
    W_md = np.ascontiguousarray(np.asarray(inputs["W_md"], np.float32))
    W_mapper = np.ascontiguousarray(np.asarray(inputs["W_mapper"], np.float32))
    UR = np.ascontiguousarray(np.asarray(inputs["user_ratings"], np.float32))
    UP = np.ascontiguousarray(np.asarray(inputs["user_personalities"], np.float32))
    top_map = np.asarray(inputs["top_map"]).astype(np.int64)
    mid_map = np.asarray(inputs["mid_map"]).astype(np.int64)

    # ---------------- kernel 1 ----------------
    nc1 = _get_k1()
    XT = np.ascontiguousarray(X.T)
    PTp = np.zeros((PD, NU), np.float32)
    PTp[:, :UP.shape[0]] = UP.T
    in_maps1 = []
    URH = UR.astype(np.float16)
    URL = (UR - URH.astype(np.float32)).astype(np.float16)
    for c in range(NCORES):
        XO = np.ascontiguousarray(X[c * 256:(c + 1) * 256, :].T)
        RSh = np.zeros((NU, SLICE), np.float16)
        RSh[:UR.shape[0], :] = URH[:, c * SLICE:(c + 1) * SLICE]
        RSl = np.zeros((NU, SLICE), np.float16)
        RSl[:UR.shape[0], :] = URL[:, c * SLICE:(c + 1) * SLICE]
        cbase = np.empty((128, 2), np.float32)
        cbase[:, 0] = c * SLICE
        cbase[:, 1] = c * SLICE + HALF
        in_maps1.append({
            "XT": XT, "PT": PTp, "XO": XO, "RSH": RSh, "RSL": RSl,
            "WSP": W_sp, "WMP": W_mp, "WSD": W_sd, "WMD": W_md, "CB": cbase,
        })
    r1 = _run_spmd(nc1, in_maps1, list(range(NCORES)))

    # ---------------- host glue ----------------
    # top: positions already global within [0,2000): map through top_map
    tpos = np.concatenate([r1[c]["TPOS"] for c in range(NCORES)], axis=0).astype(np.int64)
    top20 = top_map[tpos[:, :K]].astype(np.float32)
    # mid: merge the two 2500-halves by value
    mpos = np.concatenate([r1[c]["MPOS"] for c in range(NCORES)], axis=0).astype(np.int64)
    mval = np.concatenate([r1[c]["MVAL"] for c in range(NCORES)], axis=0)
    mpos[:, NCAND:] += MH
    mid20 = mid_map[_merge_desc(mval, mpos, K)].astype(np.float32)
    # sim: merge 8 cores x 48 candidates by value
    simv = np.concatenate([r1[c]["SIMV"] for c in range(NCORES)], axis=1)   # [2048, 384]
    simi = np.concatenate([r1[c]["SIMI"] for c in range(NCORES)], axis=1).astype(np.int64)
    sim20 = _merge_desc(simv, simi, K).astype(np.float32)
    # probs / logp (tiny fp32 matmul; same rounding class as the reference's)
    z = X @ W_mapper
    zm = z - z.max(axis=1, keepdims=True)
    e = np.exp(zm)
    probs = (e / e.sum(axis=1, keepdims=True)).astype(np.float32)
    logp = np.log(probs).astype(np.float32)
    G = _gumbel_G(B, K, 42)

    # ---------------- kernel 2 ----------------
    nc2 = _get_k2()
    con = np.zeros((128, 200), np.float32)
    con[:, 0:20] = np.arange(20)
    con[:, 20:100] = np.arange(80)
    con[:, 100:160] = np.arange(60)
    con[:, 160:166] = np.array([0, 1, 2, 0, 1, 2])
    in_maps2 = []
    for c in range(NCORES):
        r = slice(c * 256, (c + 1) * 256)
        in_maps2.append({
            "LT": np.ascontiguousarray(top20[r]),
            "LM": np.ascontiguousarray(mid20[r]),
            "LS": np.ascontiguousarray(sim20[r]),
            "LOGP": np.ascontiguousarray(logp[r]),
            "GG": np.ascontiguousarray(G[r].reshape(256, 60)),
            "CON": con,
        })
    r2 = _run_spmd(nc2, in_maps2, list(range(NCORES)))
    out = np.concatenate([r2[c]["OUT"] for c in range(NCORES)], axis=0)
    return out.astype(np.float32)


# revision 19
# speedup vs baseline: 1.0485x; 1.0026x over previous
"""Trainium2 Bass kernel for nn_EnsembleModel_61718680044080 (nms_detection).

kernel(**inputs) -> [2048, 20] float32 (fused rec lists).

Plan (8 NeuronCores, SPMD):
  Kernel 1 (heavy):
    - prior^T = user_personalities @ X^T on PE (fp32), staged via internal DRAM.
    - sim scores = prior @ user_ratings, item-sharded: each core scores its 2500-item
      slice for ALL 2048 rows (fp32 PE matmuls), then per-row top-24 of each 1250-wide
      half on the vector engine (max8/max_index/match_replace rounds) -> candidate
      values + global item ids.
    - top/mid decoder paths, batch-sharded: each core computes top_sub [256,2000] and
      mid_sub [256,5000] for its own 256 rows and emits per-row top-24 positions
      (mid in two 2500-wide halves with values, merged on host).
  Host glue (numpy only): merge candidate lists by value, map positions through
    top_map/mid_map, softmax logits, and the Gumbel noise consumed by
    jax.random.categorical in the reference (replicated bit-exactly in numpy:
    threefry2x32 splits + XLA Philox4x32-10 bit stream, rbg PRNG impl).
  Kernel 2 (fusion): batch-sharded exact replication of the reference's greedy
    set-intersection fusion + 20-step sampling scan, fully vectorized on the DVE.
"""
import numpy as np
import concourse.bass as bass
import concourse.bacc as bacc
import concourse.mybir as mybir
from concourse import tile
from concourse.bass_utils import run_bass_kernel_spmd

F32 = mybir.dt.float32
U32 = mybir.dt.uint32
ALU = mybir.AluOpType
AX = mybir.AxisListType

B = 2048          # batch
PD = 64           # p_dim
LAT = 128         # latent
NU = 2048         # users (padded 2000 -> 2048)
NT = 2000         # n_top
NM = 5000         # n_mid
MH = 2500         # mid half width
SLICE = 2500      # sim items per core
HALF = 1250       # sim half width
NCAND = 24
NEG = -3.0e38
K = 20
NCORES = 8

# ----------------------------------------------------------------------------
# numpy replication of this environment's jax.random (rbg impl):
# threefry2x32 for key/split, XLA Philox4x32-10 for bits.
# ----------------------------------------------------------------------------
_ROT = ((13, 15, 26, 6), (17, 29, 16, 24))


def _threefry2x32(k0, k1, x0, x1):
    k0 = np.asarray(k0, np.uint32); k1 = np.asarray(k1, np.uint32)
    x0 = np.asarray(x0, np.uint32).copy(); x1 = np.asarray(x1, np.uint32).copy()
    with np.errstate(over="ignore"):
        ks2 = k0 ^ k1 ^ np.uint32(0x1BD11BDA)
        ks = (k0, k1, ks2)
        x0 = x0 + ks[0]; x1 = x1 + ks[1]
        for i in range(5):
            for r in _ROT[i % 2]:
                x0 = x0 + x1
                x1 = ((x1 << np.uint32(r)) | (x1 >> np.uint32(32 - r))) ^ x0
            x0 = x0 + ks[(i + 1) % 3]
            x1 = x1 + ks[(i + 2) % 3] + np.uint32(i + 1)
    return x0, x1


_M0 = np.uint64(0xD2511F53); _M1 = np.uint64(0xCD9E8D57)
_W0 = np.uint32(0x9E3779B9); _W1 = np.uint32(0xBB67AE85)
_MASK64 = np.uint64(0xFFFFFFFF)


def _philox_block(c0, c1, c2, c3, k0, k1):
    c = [np.asarray(c0, np.uint32).copy(), np.asarray(c1, np.uint32).copy(),
         np.asarray(c2, np.uint32).copy(), np.asarray(c3, np.uint32).copy()]
    k0 = np.asarray(k0, np.uint32).copy(); k1 = np.asarray(k1, np.uint32).copy()
    with np.errstate(over="ignore"):
        for _ in range(10):
            p0 = _M0 * c[0].astype(np.uint64)
            p1 = _M1 * c[2].astype(np.uint64)
            hi0 = (p0 >> np.uint64(32)).astype(np.uint32); lo0 = (p0 & _MASK64).astype(np.uint32)
            hi1 = (p1 >> np.uint64(32)).astype(np.uint32); lo1 = (p1 & _MASK64).astype(np.uint32)
            c = [hi1 ^ c[1] ^ k0, lo1, hi0 ^ c[3] ^ k1, lo0]
            k0 = k0 + _W0; k1 = k1 + _W1
    return c


def _philox_stream(k0, k1, n_u32):
    nblk = (n_u32 + 3) // 4
    c64 = (np.uint64(k1) << np.uint64(32)) | np.uint64(k0)
    with np.errstate(over="ignore"):
        cs = c64 + np.arange(nblk, dtype=np.uint64)
    clo = (cs & _MASK64).astype(np.uint32)
    chi = (cs >> np.uint64(32)).astype(np.uint32)
    w = _philox_block(clo, chi, np.full(nblk, k0, np.uint32), np.full(nblk, k1, np.uint32),
                      np.full(nblk, k0, np.uint32), np.full(nblk, k1, np.uint32))
    out = np.empty((nblk, 4), np.uint32)
    for j in range(4):
        out[:, j] = w[j]
    return out.reshape(-1)[:n_u32]


def _gumbel_G(Bn=B, Kn=K, seed=42):
    """G[b,t,c]: gumbel noise consumed by the reference's vmapped categorical scan.
    Under vmap, the rbg impl draws each step's whole [B,3] block from ROW 0's sub key."""
    kk = np.array([(seed >> 32) & 0xFFFFFFFF, seed & 0xFFFFFFFF], np.uint32)
    y0, y1 = _threefry2x32(kk[0], kk[1], np.zeros(Bn, np.uint32), np.arange(Bn, dtype=np.uint32))
    a0, b0 = y0[0], y1[0]               # row 0's key words
    G = np.empty((Bn, Kn, 3), np.float32)
    tiny = np.float32(np.finfo(np.float32).tiny)
    onef = np.float32(1.0)
    for t in range(Kn):
        n0, n1 = _threefry2x32(a0, b0, 0, 0)
        s0, s1 = _threefry2x32(a0, b0, 0, 1)
        bits = _philox_stream(np.uint32(s0), np.uint32(s1), Bn * 3)
        u = ((bits >> np.uint32(9)) | np.uint32(0x3F800000)).view(np.float32) - onef
        u = u * (onef - tiny) + tiny
        np.maximum(u, tiny, out=u)
        G[:, t, :] = (-np.log(-np.log(u))).reshape(Bn, 3)
        a0, b0 = n0, n1
    return G


# ----------------------------------------------------------------------------
# Kernel 1 (heavy): prior^T, sim scoring + top-24, top/mid top-24 positions
# ----------------------------------------------------------------------------
def _topk24(nc, pool, s, w):
    v = pool.tile([128, NCAND], F32, tag="tk_v", name="tk_v")
    ix = pool.tile([128, NCAND], U32, tag="tk_i", name="tk_i")
    s2 = pool.tile([128, 2560], F32, tag="tk_s2", name="tk_s2")
    s3 = pool.tile([128, 2560], F32, tag="tk_s3", name="tk_s3")
    nc.vector.max(v[:, 0:8], s[:])
    nc.vector.max_index(ix[:, 0:8], v[:, 0:8], s[:])
    nc.vector.match_replace(s2[:, :w], v[:, 0:8], s[:], NEG)
    nc.vector.max(v[:, 8:16], s2[:, :w])
    nc.vector.max_index(ix[:, 8:16], v[:, 8:16], s2[:, :w])
    nc.vector.match_replace(s3[:, :w], v[:, 8:16], s2[:, :w], NEG)
    nc.vector.max(v[:, 16:24], s3[:, :w])
    nc.vector.max_index(ix[:, 16:24], v[:, 16:24], s3[:, :w])
    return v, ix


F16 = mybir.dt.float16


def _build_k1():
    nc = bacc.Bacc(None, target_bir_lowering=False)
    XT = nc.dram_tensor("XT", [PD, B], F32, kind="ExternalInput")
    PT = nc.dram_tensor("PT", [PD, NU], F32, kind="ExternalInput")
    XO = nc.dram_tensor("XO", [PD, 256], F32, kind="ExternalInput")
    RSH = nc.dram_tensor("RSH", [NU, SLICE], F16, kind="ExternalInput")
    RSL = nc.dram_tensor("RSL", [NU, SLICE], F16, kind="ExternalInput")
    WSP = nc.dram_tensor("WSP", [PD, LAT], F32, kind="ExternalInput")
    WMP = nc.dram_tensor("WMP", [PD, LAT], F32, kind="ExternalInput")
    WSD = nc.dram_tensor("WSD", [LAT, NT], F32, kind="ExternalInput")
    WMD = nc.dram_tensor("WMD", [LAT, NM], F32, kind="ExternalInput")
    CB = nc.dram_tensor("CB", [128, 2], F32, kind="ExternalInput")

    SIMV = nc.dram_tensor("SIMV", [B, 2 * NCAND], F32, kind="ExternalOutput")
    SIMI = nc.dram_tensor("SIMI", [B, 2 * NCAND], F32, kind="ExternalOutput")
    TPOS = nc.dram_tensor("TPOS", [256, NCAND], U32, kind="ExternalOutput")
    MPOS = nc.dram_tensor("MPOS", [256, 2 * NCAND], U32, kind="ExternalOutput")
    MVAL = nc.dram_tensor("MVAL", [256, 2 * NCAND], F32, kind="ExternalOutput")

    PRH = nc.dram_tensor("PRH", [NU, B], F16)   # internal: prior^T hi
    PRL = nc.dram_tensor("PRL", [NU, B], F16)   # internal: prior^T lo

    with tile.TileContext(nc) as tc:
        with (
            tc.tile_pool(name="cw", bufs=1) as cw,
            tc.tile_pool(name="tk", bufs=1) as tk,
            tc.tile_pool(name="ps", bufs=6, space="PSUM") as ps,
            tc.tile_pool(name="sbw", bufs=2) as sbw,
        ):
            cb = cw.tile([128, 2], F32, name="cb")
            nc.sync.dma_start(cb[:], CB[:])

            # Prefetch R half 0 up front; half 1 is later loaded into the SAME tiles
            # (range-level WAR lets each uc chunk reload as soon as its last half-0
            # read retires). Issued on the gpsimd SWDGE queue so the 20MB stream does
            # not clog the sync HWDGE queue that feeds stage A/C inputs and PRT writes.
            rshs = []
            rsls = []
            for uc in range(16):
                th = cw.tile([128, HALF], F16, tag=f"RH{uc}", name=f"rsh{uc}")
                tl = cw.tile([128, HALF], F16, tag=f"RL{uc}", name=f"rsl{uc}")
                nc.gpsimd.dma_start(th[:], RSH[uc * 128:(uc + 1) * 128, 0:HALF])
                nc.gpsimd.dma_start(tl[:], RSL[uc * 128:(uc + 1) * 128, 0:HALF])
                rshs.append(th)
                rsls.append(tl)

            # ---------- Stage A: PRT[u, b] = sum_p PT[p,u] * XT[p,b] ----------
            with tc.tile_pool(name="stA", bufs=1) as sa, tc.tile_pool(name="stAw", bufs=4) as saw:
                xt = sa.tile([PD, B], F32, name="xt")
                nc.sync.dma_start(xt[:], XT[:])
                pt = sa.tile([PD, NU], F32, name="pt")
                nc.sync.dma_start(pt[:], PT[:])
                for uc in range(16):
                    for bt in range(4):
                        p = ps.tile([128, 512], F32, tag="p", name="pA")
                        nc.tensor.matmul(p[:], pt[:, uc * 128:(uc + 1) * 128],
                                         xt[:, bt * 512:(bt + 1) * 512], start=True, stop=True)
                        th = saw.tile([128, 512], F16, tag="ah", name="ah")
                        nc.scalar.copy(th[:], p[:])                       # hi = fp16(prior)
                        tl32 = saw.tile([128, 512], F32, tag="al32", name="al32")
                        nc.vector.tensor_tensor(tl32[:], p[:], th[:], ALU.subtract)
                        tl = saw.tile([128, 512], F16, tag="al", name="al")
                        nc.vector.tensor_copy(tl[:], tl32[:])             # lo = fp16(prior - hi)
                        nc.sync.dma_start(PRH[uc * 128:(uc + 1) * 128, bt * 512:(bt + 1) * 512], th[:])
                        nc.sync.dma_start(PRL[uc * 128:(uc + 1) * 128, bt * 512:(bt + 1) * 512], tl[:])

            # ---------- Stage C: own-row top/mid top-24 ----------
            # Setup runs before stage B; the per-(rb,list) units are emitted
            # interleaved between early stage-B iterations (CPOINTS) so their
            # PSUM-copy/top-k latency hides under stage B's matmul stream.
            sccm = tc.tile_pool(name="stC", bufs=1)
            scp = sccm.__enter__()
            scwcm = tc.tile_pool(name="stCw", bufs=2)
            scw = scwcm.__enter__()
            xo = scp.tile([PD, 256], F32, name="xo")
            nc.sync.dma_start(xo[:], XO[:])
            wsp = scp.tile([PD, LAT], F32, name="wsp")
            nc.sync.dma_start(wsp[:], WSP[:])
            wmp = scp.tile([PD, LAT], F32, name="wmp")
            nc.sync.dma_start(wmp[:], WMP[:])
            wsd = scp.tile([LAT, NT], F32, name="wsd")
            nc.sync.dma_start(wsd[:], WSD[:])
            wmd = scp.tile([LAT, NM], F32, name="wmd")
            nc.sync.dma_start(wmd[:], WMD[:])

            ph1 = ps.tile([128, 512], F32, tag="p", name="ph1")
            nc.tensor.matmul(ph1[:, :256], wsp[:], xo[:], start=True, stop=True)
            h1t = scp.tile([LAT, 256], F32, name="h1t")
            nc.scalar.copy(h1t[:], ph1[:, :256])
            ph2 = ps.tile([128, 512], F32, tag="p", name="ph2")
            nc.tensor.matmul(ph2[:, :256], wmp[:], xo[:], start=True, stop=True)
            h1m = scp.tile([LAT, 256], F32, name="h1m")
            nc.scalar.copy(h1m[:], ph2[:, :256])

            def c_top(rb):
                tsub = scw.tile([128, NT], F32, tag="sub", name="tsub")
                for nt in range(4):
                    w = min(512, NT - nt * 512)
                    p = ps.tile([128, 512], F32, tag="p", name="pC")
                    nc.tensor.matmul(p[:, :w], h1t[:, rb * 128:(rb + 1) * 128],
                                     wsd[:, nt * 512: nt * 512 + w], start=True, stop=True)
                    nc.scalar.copy(tsub[:, nt * 512: nt * 512 + w], p[:, :w])
                _, ixt = _topk24(nc, tk, tsub, NT)
                nc.sync.dma_start(TPOS[rb * 128:(rb + 1) * 128, :], ixt[:])

            def c_mid(rb, mh):
                msub = scw.tile([128, MH], F32, tag="sub", name="msub")
                for nt in range(5):
                    w = min(512, MH - nt * 512)
                    p = ps.tile([128, 512], F32, tag="p", name="pM")
                    nc.tensor.matmul(p[:, :w], h1m[:, rb * 128:(rb + 1) * 128],
                                     wmd[:, mh * MH + nt * 512: mh * MH + nt * 512 + w],
                                     start=True, stop=True)
                    nc.scalar.copy(msub[:, nt * 512: nt * 512 + w], p[:, :w])
                vm, ixm = _topk24(nc, tk, msub, MH)
                nc.sync.dma_start(MPOS[rb * 128:(rb + 1) * 128, mh * NCAND:(mh + 1) * NCAND], ixm[:])
                nc.sync.dma_start(MVAL[rb * 128:(rb + 1) * 128, mh * NCAND:(mh + 1) * NCAND], vm[:])

            C_UNITS = [lambda: c_top(0), lambda: c_mid(0, 0), lambda: c_mid(0, 1),
                       lambda: c_top(1), lambda: c_mid(1, 0), lambda: c_mid(1, 1)]
            CPOINTS = {1: 0, 3: 1, 5: 2, 7: 3, 9: 4, 11: 5}

            def emit_c(i):
                C_UNITS[i]()

            # ---------- Stage B: sim scoring (fp16 hi/lo split, 3 matmuls/chunk) + top-24 ----------
            def sim_bc(h, bc, rsh, rsl):
                if True:
                    pbh = sbw.tile([128, 16 * 128], F16, tag="pbh", name="pbh")
                    nc.sync.dma_start(pbh[:], bass.AP(PRH, bc * 128, [[B, 128], [128 * B, 16], [1, 128]]))
                    pbl = sbw.tile([128, 16 * 128], F16, tag="pbl", name="pbl")
                    nc.sync.dma_start(pbl[:], bass.AP(PRL, bc * 128, [[B, 128], [128 * B, 16], [1, 128]]))
                    scores = sbw.tile([128, HALF], F32, tag="scores", name="scores")
                    for nt, w in ((0, 512), (512, 512), (1024, 226)):
                        p = ps.tile([128, 512], F32, tag="p", name="pB")
                        for uc in range(16):
                            first = uc == 0
                            last = uc == 15
                            nc.tensor.matmul(p[:, :w], pbh[:, uc * 128:(uc + 1) * 128],
                                             rsh[uc][:, nt: nt + w], start=first, stop=False)
                            nc.tensor.matmul(p[:, :w], pbh[:, uc * 128:(uc + 1) * 128],
                                             rsl[uc][:, nt: nt + w], start=False, stop=False)
                            nc.tensor.matmul(p[:, :w], pbl[:, uc * 128:(uc + 1) * 128],
                                             rsh[uc][:, nt: nt + w], start=False, stop=last)
                        nc.scalar.copy(scores[:, nt:nt + w], p[:, :w])
                    v24, ix24 = _topk24(nc, tk, scores, HALF)
                    idsf = tk.tile([128, NCAND], F32, tag="idsf", name="idsf")
                    nc.vector.tensor_copy(idsf[:], ix24[:])
                    nc.vector.tensor_scalar_add(idsf[:], idsf[:], cb[:, h:h + 1])
                    nc.sync.dma_start(SIMV[bc * 128:(bc + 1) * 128, h * NCAND:(h + 1) * NCAND], v24[:])
                    nc.sync.dma_start(SIMI[bc * 128:(bc + 1) * 128, h * NCAND:(h + 1) * NCAND], idsf[:])

            for bc in range(16):
                sim_bc(0, bc, rshs, rsls)
                if bc in CPOINTS:
                    emit_c(CPOINTS[bc])
            for uc in range(16):
                nc.gpsimd.dma_start(rshs[uc][:], RSH[uc * 128:(uc + 1) * 128, HALF:2 * HALF])
                nc.gpsimd.dma_start(rsls[uc][:], RSL[uc * 128:(uc + 1) * 128, HALF:2 * HALF])
            for bc in range(16):
                sim_bc(1, bc, rshs, rsls)
            scwcm.__exit__(None, None, None)
            sccm.__exit__(None, None, None)
    nc.compile()
    return nc


# ----------------------------------------------------------------------------
# Kernel 2: exact fusion scan (see module docstring of the reference _fuse_one)
# ----------------------------------------------------------------------------
def _build_k2():
    nc = bacc.Bacc(None, target_bir_lowering=False)
    LT = nc.dram_tensor("LT", [256, K], F32, kind="ExternalInput")
    LM = nc.dram_tensor("LM", [256, K], F32, kind="ExternalInput")
    LS = nc.dram_tensor("LS", [256, K], F32, kind="ExternalInput")
    LOGP = nc.dram_tensor("LOGP", [256, 3], F32, kind="ExternalInput")
    GG = nc.dram_tensor("GG", [256, 60], F32, kind="ExternalInput")
    CON = nc.dram_tensor("CON", [128, 200], F32, kind="ExternalInput")
    OUT = nc.dram_tensor("OUT", [256, K], F32, kind="ExternalOutput")

    def v(tl, dims, off=0):
        base = tl[:]
        return bass.AP(base.tensor, base.offset + off, [list(base.ap[0])] + [list(d) for d in dims])

    with tile.TileContext(nc) as tc:
        with tc.tile_pool(name="p", bufs=1) as P:
            def T(w, name):
                return P.tile([128, w], F32, tag=name, name=name)

            t = T(40, "t"); m = T(40, "m"); s = T(40, "s")
            for blk in range(2):
                nc.sync.dma_start(t[:, blk * 20:(blk + 1) * 20], LT[blk * 128:(blk + 1) * 128, :])
                nc.sync.dma_start(m[:, blk * 20:(blk + 1) * 20], LM[blk * 128:(blk + 1) * 128, :])
                nc.sync.dma_start(s[:, blk * 20:(blk + 1) * 20], LS[blk * 128:(blk + 1) * 128, :])
            lp = T(6, "lp")
            for blk in range(2):
                nc.sync.dma_start(lp[:, blk * 3:(blk + 1) * 3], LOGP[blk * 128:(blk + 1) * 128, :])
            g = T(120, "g")
            for blk in range(2):
                nc.sync.dma_start(g[:, blk * 60:(blk + 1) * 60], GG[blk * 128:(blk + 1) * 128, :])
            con = T(200, "con")
            nc.sync.dma_start(con[:], CON[:])

            eq800 = T(800, "eq800")

            def isin(out, a, b, bmask=None):
                eq_v = v(eq800, [[400, 2], [20, 20], [1, 20]])
                a_v = v(a, [[20, 2], [1, 20], [0, 20]])
                b_v = v(b, [[20, 2], [0, 20], [1, 20]])
                nc.vector.tensor_tensor(eq_v, a_v, b_v, ALU.is_equal)
                if bmask is not None:
                    bm_v = v(bmask, [[20, 2], [0, 20], [1, 20]])
                    nc.vector.tensor_tensor(eq_v, eq_v, bm_v, ALU.mult)
                nc.vector.tensor_reduce(v(out, [[20, 2], [1, 20]]),
                                        v(eq800, [[400, 2], [20, 20], [1, 20]]), AX.X, ALU.max)

            def tnot(out, a):
                nc.vector.tensor_scalar(out[:], a[:], -1.0, 1.0, ALU.mult, ALU.add)

            def tmul(out, a, b):
                nc.vector.tensor_tensor(out[:], a[:], b[:], ALU.mult)

            mtm = T(40, "mtm"); mts = T(40, "mts"); mms = T(40, "mms")
            isin(mtm, t, m); isin(mts, t, s); isin(mms, m, s)
            mc = T(40, "mc")
            tmul(mc, mtm, mts); tmul(mc, mc, mms)
            icm = T(40, "icm")
            isin(icm, m, t, bmask=mc)
            ntc = T(40, "ntc"); tnot(ntc, mc)
            mtm2 = T(40, "mtm2"); tmul(mtm2, mtm, ntc)
            mts2 = T(40, "mts2"); tmul(mts2, mts, ntc)
            nicm = T(40, "nicm"); tnot(nicm, icm)
            mms2 = T(40, "mms2"); tmul(mms2, mms, nicm)
            tmp = T(40, "tmpa"); tmp2 = T(40, "tmpb")
            tpm = T(40, "tpm")
            tnot(tmp, mtm2); tnot(tmp2, mts2)
            tmul(tpm, ntc, tmp); tmul(tpm, tpm, tmp2)
            mpm = T(40, "mpm")
            isin(tmp, m, t, bmask=mtm2)
            tnot(tmp, tmp); tnot(tmp2, mms2)
            tmul(mpm, nicm, tmp); tmul(mpm, mpm, tmp2)
            spm = T(40, "spm")
            isin(tmp, s, t, bmask=mc); tnot(tmp, tmp)
            isin(tmp2, s, t, bmask=mts2); tnot(tmp2, tmp2)
            tmul(spm, tmp, tmp2)
            isin(tmp, s, m, bmask=mms2); tnot(tmp, tmp)
            tmul(spm, spm, tmp)

            # ---- det list ----
            dv = T(160, "dv"); dm = T(160, "dm")
            for j, srcm in enumerate((t, t, t, m)):
                nc.vector.tensor_copy(v(dv, [[80, 2], [1, 20]], j * 20), v(srcm, [[20, 2], [1, 20]]))
            for j, srcm in enumerate((mc, mtm2, mts2, mms2)):
                nc.vector.tensor_copy(v(dm, [[80, 2], [1, 20]], j * 20), v(srcm, [[20, 2], [1, 20]]))
            zz = T(80, "zz"); nc.vector.memset(zz[:], 0.0)
            cs = T(160, "cs")
            for blk in range(2):
                nc.vector.tensor_tensor_scan(cs[:, blk * 80:(blk + 1) * 80],
                                             dm[:, blk * 80:(blk + 1) * 80], zz[:],
                                             0.0, ALU.add, ALU.add)
            nd = T(2, "nd")
            nc.vector.tensor_copy(v(nd, [[1, 2]]), v(cs, [[80, 2]], 79))
            pos = T(160, "pos"); csm1 = T(160, "csm1")
            nc.vector.tensor_tensor(pos[:], v(con, [[0, 2], [1, 80]], 20), v(cs, [[80, 2], [1, 80]]), ALU.subtract)
            nc.vector.tensor_tensor(pos[:], pos[:], v(nd, [[1, 2], [0, 80]]), ALU.add)
            nc.vector.tensor_scalar(csm1[:], cs[:], -1.0, None, ALU.add)
            nc.vector.tensor_tensor(csm1[:], csm1[:], pos[:], ALU.subtract)
            nc.vector.tensor_tensor(csm1[:], csm1[:], dm[:], ALU.mult)
            nc.vector.tensor_tensor(pos[:], pos[:], csm1[:], ALU.add)
            E = T(3200, "E")
            E_v = v(E, [[1600, 2], [80, 20], [1, 80]])
            nc.vector.tensor_tensor(E_v, v(con, [[0, 2], [1, 20], [0, 80]], 0),
                                    v(pos, [[80, 2], [0, 20], [1, 80]]), ALU.is_equal)
            nc.vector.tensor_tensor(E_v, E_v, v(dv, [[80, 2], [0, 20], [1, 80]]), ALU.mult)
            det20 = T(40, "det20")
            nc.vector.tensor_reduce(v(det20, [[20, 2], [1, 20]]),
                                    v(E, [[1600, 2], [80, 20], [1, 80]]), AX.X, ALU.add)

            # ---- pools ----
            pv = T(120, "pv"); pm = T(120, "pm")
            for li, srcm in enumerate((t, m, s)):
                nc.vector.tensor_copy(v(pv, [[60, 2], [1, 20]], li * 20), v(srcm, [[20, 2], [1, 20]]))
            for li, srcm in enumerate((tpm, mpm, spm)):
                nc.vector.tensor_copy(v(pm, [[60, 2], [1, 20]], li * 20), v(srcm, [[20, 2], [1, 20]]))
            csp = T(120, "csp")
            for bl in range(6):
                nc.vector.tensor_tensor_scan(csp[:, bl * 20:(bl + 1) * 20],
                                             pm[:, bl * 20:(bl + 1) * 20], zz[:, :20],
                                             0.0, ALU.add, ALU.add)
            cnt = T(6, "cnt")
            nc.vector.tensor_copy(v(cnt, [[1, 6]]), v(csp, [[20, 6]], 19))
            pp = T(120, "pp"); cspm1 = T(120, "cspm1")
            nc.vector.tensor_tensor(pp[:], v(con, [[0, 6], [1, 20]], 0), v(csp, [[20, 6], [1, 20]]), ALU.subtract)
            nc.vector.tensor_tensor(pp[:], pp[:], v(cnt, [[1, 6], [0, 20]]), ALU.add)
            nc.vector.tensor_scalar(cspm1[:], csp[:], -1.0, None, ALU.add)
            nc.vector.tensor_tensor(cspm1[:], cspm1[:], pp[:], ALU.subtract)
            nc.vector.tensor_tensor(cspm1[:], cspm1[:], pm[:], ALU.mult)
            nc.vector.tensor_tensor(pp[:], pp[:], cspm1[:], ALU.add)
            E2 = T(2400, "E2")
            E2_v = v(E2, [[400, 6], [20, 20], [1, 20]])
            nc.vector.tensor_tensor(E2_v, v(con, [[0, 6], [1, 20], [0, 20]], 0),
                                    v(pp, [[20, 6], [0, 20], [1, 20]]), ALU.is_equal)
            nc.vector.tensor_tensor(E2_v, E2_v, v(pv, [[20, 6], [0, 20], [1, 20]]), ALU.mult)
            pc = T(120, "pc")
            nc.vector.tensor_reduce(v(pc, [[20, 6], [1, 20]]),
                                    v(E2, [[400, 6], [20, 20], [1, 20]]), AX.X, ALU.add)

            # ---- fusion scan ----
            sct = T(6, "sct")
            ptrA = T(6, "ptrA"); ptrB = T(6, "ptrB")
            nc.vector.memset(ptrA[:], 0.0)
            outb = T(40, "outb")
            av = T(6, "av"); sc2 = T(6, "sc2"); mx = T(2, "mx"); eqm = T(6, "eqm")
            i1 = T(2, "i1"); i2 = T(2, "i2"); idx = T(2, "idx"); anyav = T(2, "anyav")
            oh = T(6, "oh"); pm6 = T(6, "pm6"); psel = T(2, "psel"); tgt = T(2, "tgt")
            sel = T(120, "sel"); sp = T(120, "sp"); samp = T(2, "samp")
            used = T(2, "used"); d1 = T(2, "d1"); d2 = T(2, "d2"); val = T(2, "val")
            ndet = T(2, "ndet"); dp = T(6, "dp")
            for tstep in range(K):
                ptr, ptrn = (ptrA, ptrB) if tstep % 2 == 0 else (ptrB, ptrA)
                nc.vector.tensor_tensor(av[:], ptr[:], cnt[:], ALU.is_lt)
                nc.vector.tensor_tensor(sc2[:], lp[:], av[:], ALU.mult)
                nc.vector.tensor_scalar(sct[:], av[:], 3.0e38, -3.0e38, ALU.mult, ALU.add)
                nc.vector.tensor_tensor(sc2[:], sc2[:], sct[:], ALU.add)
                nc.vector.tensor_tensor(sc2[:], sc2[:], v(g, [[60, 2], [1, 3]], tstep * 3), ALU.add)
                nc.vector.tensor_reduce(v(mx, [[1, 2]]), v(sc2, [[3, 2], [1, 3]]), AX.X, ALU.max)
                nc.vector.tensor_tensor(eqm[:], sc2[:], v(mx, [[1, 2], [0, 3]]), ALU.is_equal)
                nc.vector.tensor_scalar(i1[:], v(eqm, [[3, 2]], 0), -1.0, 1.0, ALU.mult, ALU.add)
                nc.vector.tensor_scalar(i2[:], v(eqm, [[3, 2]], 1), -1.0, 2.0, ALU.mult, ALU.add)
                nc.vector.tensor_tensor(idx[:], i1[:], i2[:], ALU.mult)
                nc.vector.tensor_reduce(v(anyav, [[1, 2]]), v(av, [[3, 2], [1, 3]]), AX.X, ALU.max)
                nc.vector.tensor_tensor(idx[:], idx[:], anyav[:], ALU.mult)
                nc.vector.tensor_tensor(oh[:], v(con, [[1, 6]], 160), v(idx, [[1, 2], [0, 3]]), ALU.is_equal)
                nc.vector.tensor_tensor(pm6[:], ptr[:], oh[:], ALU.mult)
                nc.vector.tensor_reduce(v(psel, [[1, 2]]), v(pm6, [[3, 2], [1, 3]]), AX.X, ALU.add)
                nc.vector.tensor_scalar(psel[:], psel[:], float(K - 1), None, ALU.min)
                nc.vector.tensor_scalar(tgt[:], idx[:], 20.0, None, ALU.mult)
                nc.vector.tensor_tensor(tgt[:], tgt[:], psel[:], ALU.add)
                nc.vector.tensor_tensor(sel[:], v(con, [[0, 2], [1, 60]], 100),
                                        v(tgt, [[1, 2], [0, 60]]), ALU.is_equal)
                nc.vector.tensor_tensor(sp[:], sel[:], pc[:], ALU.mult)
                nc.vector.tensor_reduce(v(samp, [[1, 2]]), v(sp, [[60, 2], [1, 60]]), AX.X, ALU.add)
                nc.vector.tensor_scalar(used[:], nd[:], float(tstep), None, ALU.is_gt)
                nc.vector.tensor_tensor(d1[:], v(det20, [[20, 2]], tstep), samp[:], ALU.subtract)
                nc.vector.tensor_tensor(d2[:], used[:], d1[:], ALU.mult)
                nc.vector.tensor_tensor(val[:], samp[:], d2[:], ALU.add)
                nc.vector.tensor_copy(v(outb, [[20, 2]], tstep), val[:])
                nc.vector.tensor_scalar(ndet[:], used[:], -1.0, 1.0, ALU.mult, ALU.add)
                nc.vector.tensor_tensor(dp[:], oh[:], v(ndet, [[1, 2], [0, 3]]), ALU.mult)
                nc.vector.tensor_tensor(ptrn[:], ptr[:], dp[:], ALU.add)
            for blk in range(2):
                nc.sync.dma_start(OUT[blk * 128:(blk + 1) * 128, :], outb[:, blk * 20:(blk + 1) * 20])
    nc.compile()
    return nc


_K1 = None
_K2 = None


def _run_spmd(nc, in_maps, core_ids):
    """run_bass_kernel_spmd with retries: transient NRT device errors
    (e.g. NRT_EXEC_UNIT_UNRECOVERABLE right after another process released
    the cores) usually succeed on the next attempt."""
    last = None
    for _ in range(3):
        try:
            return run_bass_kernel_spmd(nc, in_maps, core_ids).results
        except Exception as e:   # noqa: BLE001
            last = e
            import time
            time.sleep(2.0)
    raise last


def _get_k1():
    global _K1
    if _K1 is None:
        _K1 = _build_k1()
    return _K1


def _get_k2():
    global _K2
    if _K2 is None:
        _K2 = _build_k2()
    return _K2


def _merge_desc(vals, ids, k):
    """Per-row top-k by value desc, ties broken by ascending id (jax.lax.top_k semantics)."""
    order = np.lexsort((ids, -vals.astype(np.float64)), axis=-1)[:, :k]
    return np.take_along_axis(ids, order, axis=1)


def kernel(**inputs):
    X = np.ascontiguousarray(np.asarray(inputs["X"], np.float32))
    W_sp = np.ascontiguousarray(np.asarray(inputs["W_sp"], np.float32))
    W_sd = np.ascontiguousarray(np.asarray(inputs["W_sd"], np.float32))
    W_mp = np.ascontiguousarray(np.asarray(inputs["W_mp"], np.float32))
    W_md = np.ascontiguousarray(np.asarray(inputs["W_md"], np.float32))
    W_mapper = np.ascontiguousarray(np.asarray(inputs["W_mapper"], np.float32))
    UR = np.ascontiguousarray(np.asarray(inputs["user_ratings"], np.float32))
    UP = np.ascontiguousarray(np.asarray(inputs["user_personalities"], np.float32))
    top_map = np.asarray(inputs["top_map"]).astype(np.int64)
    mid_map = np.asarray(inputs["mid_map"]).astype(np.int64)

    # ---------------- kernel 1 ----------------
    nc1 = _get_k1()
    XT = np.ascontiguousarray(X.T)
    PTp = np.zeros((PD, NU), np.float32)
    PTp[:, :UP.shape[0]] = UP.T
    in_maps1 = []
    URH = UR.astype(np.float16)
    URL = (UR - URH.astype(np.float32)).astype(np.float16)
    for c in range(NCORES):
        XO = np.ascontiguousarray(X[c * 256:(c + 1) * 256, :].T)
        RSh = np.zeros((NU, SLICE), np.float16)
        RSh[:UR.shape[0], :] = URH[:, c * SLICE:(c + 1) * SLICE]
        RSl = np.zeros((NU, SLICE), np.float16)
        RSl[:UR.shape[0], :] = URL[:, c * SLICE:(c + 1) * SLICE]
        cbase = np.empty((128, 2), np.float32)
        cbase[:, 0] = c * SLICE
        cbase[:, 1] = c * SLICE + HALF
        in_maps1.append({
            "XT": XT, "PT": PTp, "XO": XO, "RSH": RSh, "RSL": RSl,
            "WSP": W_sp, "WMP": W_mp, "WSD": W_sd, "WMD": W_md, "CB": cbase,
        })
    r1 = _run_spmd(nc1, in_maps1, list(range(NCORES)))

    # ---------------- host glue ----------------
    # top: positions already global within [0,2000): map through top_map
    tpos = np.concatenate([r1[c]["TPOS"] for c in range(NCORES)], axis=0).astype(np.int64)
    top20 = top_map[tpos[:, :K]].astype(np.float32)
    # mid: merge the two 2500-halves by value
    mpos = np.concatenate([r1[c]["MPOS"] for c in range(NCORES)], axis=0).astype(np.int64)
    mval = np.concatenate([r1[c]["MVAL"] for c in range(NCORES)], axis=0)
    mpos[:, NCAND:] += MH
    mid20 = mid_map[_merge_desc(mval, mpos, K)].astype(np.float32)
    # sim: merge 8 cores x 48 candidates by value
    simv = np.concatenate([r1[c]["SIMV"] for c in range(NCORES)], axis=1)   # [2048, 384]
    simi = np.concatenate([r1[c]["SIMI"] for c in range(NCORES)], axis=1).astype(np.int64)
    sim20 = _merge_desc(simv, simi, K).astype(np.float32)
    # probs / logp (tiny fp32 matmul; same rounding class as the reference's)
    z = X @ W_mapper
    zm = z - z.max(axis=1, keepdims=True)
    e = np.exp(zm)
    probs = (e / e.sum(axis=1, keepdims=True)).astype(np.float32)
    logp = np.log(probs).astype(np.float32)
    G = _gumbel_G(B, K, 42)

    # ---------------- kernel 2 ----------------
    nc2 = _get_k2()
    con = np.zeros((128, 200), np.float32)
    con[:, 0:20] = np.arange(20)
    con[:, 20:100] = np.arange(80)
    con[:, 100:160] = np.arange(60)
    con[:, 160:166] = np.array([0, 1, 2, 0, 1, 2])
    in_maps2 = []
    for c in range(NCORES):
        r = slice(c * 256, (c + 1) * 256)
        in_maps2.append({
            "LT": np.ascontiguousarray(top20[r]),
            "LM": np.ascontiguousarray(mid20[r]),
            "LS": np.ascontiguousarray(sim20[r]),
            "LOGP": np.ascontiguousarray(logp[r]),
            "GG": np.ascontiguousarray(G[r].reshape(256, 60)),
            "CON": con,
        })
    r2 = _run_spmd(nc2, in_maps2, list(range(NCORES)))
    out = np.concatenate([r2[c]["OUT"] for c in range(NCORES)], axis=0)
    return out.astype(np.float32)


# revision 20
# speedup vs baseline: 1.0744x; 1.0247x over previous
"""Trainium2 Bass kernel for nn_EnsembleModel_61718680044080 (nms_detection).

kernel(**inputs) -> [2048, 20] float32 (fused rec lists).

Plan (8 NeuronCores, SPMD):
  Kernel 1 (heavy):
    - prior^T = user_personalities @ X^T on PE (fp32), staged via internal DRAM.
    - sim scores = prior @ user_ratings, item-sharded: each core scores its 2500-item
      slice for ALL 2048 rows (fp32 PE matmuls), then per-row top-24 of each 1250-wide
      half on the vector engine (max8/max_index/match_replace rounds) -> candidate
      values + global item ids.
    - top/mid decoder paths, batch-sharded: each core computes top_sub [256,2000] and
      mid_sub [256,5000] for its own 256 rows and emits per-row top-24 positions
      (mid in two 2500-wide halves with values, merged on host).
  Host glue (numpy only): merge candidate lists by value, map positions through
    top_map/mid_map, softmax logits, and the Gumbel noise consumed by
    jax.random.categorical in the reference (replicated bit-exactly in numpy:
    threefry2x32 splits + XLA Philox4x32-10 bit stream, rbg PRNG impl).
  Kernel 2 (fusion): batch-sharded exact replication of the reference's greedy
    set-intersection fusion + 20-step sampling scan, fully vectorized on the DVE.
"""
import numpy as np
import concourse.bass as bass
import concourse.bacc as bacc
import concourse.mybir as mybir
from concourse import tile
from concourse.bass_utils import run_bass_kernel_spmd

F32 = mybir.dt.float32
U32 = mybir.dt.uint32
ALU = mybir.AluOpType
AX = mybir.AxisListType

B = 2048          # batch
PD = 64           # p_dim
LAT = 128         # latent
NU = 2048         # users (padded 2000 -> 2048)
NT = 2000         # n_top
NM = 5000         # n_mid
MH = 2500         # mid half width
SLICE = 2500      # sim items per core
HALF = 1250       # sim half width
NCAND = 24
NEG = -3.0e38
K = 20
NCORES = 8

# ----------------------------------------------------------------------------
# numpy replication of this environment's jax.random (rbg impl):
# threefry2x32 for key/split, XLA Philox4x32-10 for bits.
# ----------------------------------------------------------------------------
_ROT = ((13, 15, 26, 6), (17, 29, 16, 24))


def _threefry2x32(k0, k1, x0, x1):
    k0 = np.asarray(k0, np.uint32); k1 = np.asarray(k1, np.uint32)
    x0 = np.asarray(x0, np.uint32).copy(); x1 = np.asarray(x1, np.uint32).copy()
    with np.errstate(over="ignore"):
        ks2 = k0 ^ k1 ^ np.uint32(0x1BD11BDA)
        ks = (k0, k1, ks2)
        x0 = x0 + ks[0]; x1 = x1 + ks[1]
        for i in range(5):
            for r in _ROT[i % 2]:
                x0 = x0 + x1
                x1 = ((x1 << np.uint32(r)) | (x1 >> np.uint32(32 - r))) ^ x0
            x0 = x0 + ks[(i + 1) % 3]
            x1 = x1 + ks[(i + 2) % 3] + np.uint32(i + 1)
    return x0, x1


_M0 = np.uint64(0xD2511F53); _M1 = np.uint64(0xCD9E8D57)
_W0 = np.uint32(0x9E3779B9); _W1 = np.uint32(0xBB67AE85)
_MASK64 = np.uint64(0xFFFFFFFF)


def _philox_block(c0, c1, c2, c3, k0, k1):
    c = [np.asarray(c0, np.uint32).copy(), np.asarray(c1, np.uint32).copy(),
         np.asarray(c2, np.uint32).copy(), np.asarray(c3, np.uint32).copy()]
    k0 = np.asarray(k0, np.uint32).copy(); k1 = np.asarray(k1, np.uint32).copy()
    with np.errstate(over="ignore"):
        for _ in range(10):
            p0 = _M0 * c[0].astype(np.uint64)
            p1 = _M1 * c[2].astype(np.uint64)
            hi0 = (p0 >> np.uint64(32)).astype(np.uint32); lo0 = (p0 & _MASK64).astype(np.uint32)
            hi1 = (p1 >> np.uint64(32)).astype(np.uint32); lo1 = (p1 & _MASK64).astype(np.uint32)
            c = [hi1 ^ c[1] ^ k0, lo1, hi0 ^ c[3] ^ k1, lo0]
            k0 = k0 + _W0; k1 = k1 + _W1
    return c


def _philox_stream(k0, k1, n_u32):
    nblk = (n_u32 + 3) // 4
    c64 = (np.uint64(k1) << np.uint64(32)) | np.uint64(k0)
    with np.errstate(over="ignore"):
        cs = c64 + np.arange(nblk, dtype=np.uint64)
    clo = (cs & _MASK64).astype(np.uint32)
    chi = (cs >> np.uint64(32)).astype(np.uint32)
    w = _philox_block(clo, chi, np.full(nblk, k0, np.uint32), np.full(nblk, k1, np.uint32),
                      np.full(nblk, k0, np.uint32), np.full(nblk, k1, np.uint32))
    out = np.empty((nblk, 4), np.uint32)
    for j in range(4):
        out[:, j] = w[j]
    return out.reshape(-1)[:n_u32]


def _gumbel_G(Bn=B, Kn=K, seed=42):
    """G[b,t,c]: gumbel noise consumed by the reference's vmapped categorical scan.
    Under vmap, the rbg impl draws each step's whole [B,3] block from ROW 0's sub key."""
    kk = np.array([(seed >> 32) & 0xFFFFFFFF, seed & 0xFFFFFFFF], np.uint32)
    y0, y1 = _threefry2x32(kk[0], kk[1], np.zeros(Bn, np.uint32), np.arange(Bn, dtype=np.uint32))
    a0, b0 = y0[0], y1[0]               # row 0's key words
    G = np.empty((Bn, Kn, 3), np.float32)
    tiny = np.float32(np.finfo(np.float32).tiny)
    onef = np.float32(1.0)
    for t in range(Kn):
        n0, n1 = _threefry2x32(a0, b0, 0, 0)
        s0, s1 = _threefry2x32(a0, b0, 0, 1)
        bits = _philox_stream(np.uint32(s0), np.uint32(s1), Bn * 3)
        u = ((bits >> np.uint32(9)) | np.uint32(0x3F800000)).view(np.float32) - onef
        u = u * (onef - tiny) + tiny
        np.maximum(u, tiny, out=u)
        G[:, t, :] = (-np.log(-np.log(u))).reshape(Bn, 3)
        a0, b0 = n0, n1
    return G


# ----------------------------------------------------------------------------
# Kernel 1 (heavy): prior^T, sim scoring + top-24, top/mid top-24 positions
# ----------------------------------------------------------------------------
def _topk24(nc, pool, s, w):
    v = pool.tile([128, NCAND], F32, tag="tk_v", name="tk_v")
    ix = pool.tile([128, NCAND], U32, tag="tk_i", name="tk_i")
    s2 = pool.tile([128, 2560], F32, tag="tk_s2", name="tk_s2")
    s3 = pool.tile([128, 2560], F32, tag="tk_s3", name="tk_s3")
    nc.vector.max(v[:, 0:8], s[:])
    nc.vector.max_index(ix[:, 0:8], v[:, 0:8], s[:])
    nc.vector.match_replace(s2[:, :w], v[:, 0:8], s[:], NEG)
    nc.vector.max(v[:, 8:16], s2[:, :w])
    nc.vector.max_index(ix[:, 8:16], v[:, 8:16], s2[:, :w])
    nc.vector.match_replace(s3[:, :w], v[:, 8:16], s2[:, :w], NEG)
    nc.vector.max(v[:, 16:24], s3[:, :w])
    nc.vector.max_index(ix[:, 16:24], v[:, 16:24], s3[:, :w])
    return v, ix


F16 = mybir.dt.float16


def _build_k1():
    nc = bacc.Bacc(None, target_bir_lowering=False)
    XT = nc.dram_tensor("XT", [PD, B], F32, kind="ExternalInput")
    PT = nc.dram_tensor("PT", [PD, NU], F32, kind="ExternalInput")
    XO = nc.dram_tensor("XO", [PD, 256], F32, kind="ExternalInput")
    RSH = nc.dram_tensor("RSH", [NU, SLICE], F16, kind="ExternalInput")
    RSL = nc.dram_tensor("RSL", [NU, SLICE], F16, kind="ExternalInput")
    WSP = nc.dram_tensor("WSP", [PD, LAT], F32, kind="ExternalInput")
    WMP = nc.dram_tensor("WMP", [PD, LAT], F32, kind="ExternalInput")
    WSD = nc.dram_tensor("WSD", [LAT, NT], F32, kind="ExternalInput")
    WMD = nc.dram_tensor("WMD", [LAT, NM], F32, kind="ExternalInput")
    CB = nc.dram_tensor("CB", [128, 2], F32, kind="ExternalInput")

    SIMV = nc.dram_tensor("SIMV", [B, 2 * NCAND], F32, kind="ExternalOutput")
    SIMI = nc.dram_tensor("SIMI", [B, 2 * NCAND], F32, kind="ExternalOutput")
    TPOS = nc.dram_tensor("TPOS", [256, NCAND], U32, kind="ExternalOutput")
    MPOS = nc.dram_tensor("MPOS", [256, 2 * NCAND], U32, kind="ExternalOutput")
    MVAL = nc.dram_tensor("MVAL", [256, 2 * NCAND], F32, kind="ExternalOutput")

    PRH = nc.dram_tensor("PRH", [NU, B], F16)   # internal: prior^T hi
    PRL = nc.dram_tensor("PRL", [NU, B], F16)   # internal: prior^T lo

    with tile.TileContext(nc) as tc:
        with (
            tc.tile_pool(name="cw", bufs=1) as cw,
            tc.tile_pool(name="tk", bufs=1) as tk,
            tc.tile_pool(name="ps", bufs=6, space="PSUM") as ps,
            tc.tile_pool(name="sbw", bufs=2) as sbw,
        ):
            cb = cw.tile([128, 2], F32, name="cb")
            nc.sync.dma_start(cb[:], CB[:])

            # Prefetch R half 0 up front; half 1 is later loaded into the SAME tiles
            # (range-level WAR lets each uc chunk reload as soon as its last half-0
            # read retires). Issued on the gpsimd SWDGE queue so the 20MB stream does
            # not clog the sync HWDGE queue that feeds stage A/C inputs and PRT writes.
            rshs = []
            rsls = []
            for uc in range(16):
                th = cw.tile([128, HALF], F16, tag=f"RH{uc}", name=f"rsh{uc}")
                tl = cw.tile([128, HALF], F16, tag=f"RL{uc}", name=f"rsl{uc}")
                nc.gpsimd.dma_start(th[:], RSH[uc * 128:(uc + 1) * 128, 0:HALF])
                nc.gpsimd.dma_start(tl[:], RSL[uc * 128:(uc + 1) * 128, 0:HALF])
                rshs.append(th)
                rsls.append(tl)

            # ---------- Stage A: PRT[u, b] = sum_p PT[p,u] * XT[p,b] ----------
            with tc.tile_pool(name="stA", bufs=1) as sa, tc.tile_pool(name="stAw", bufs=4) as saw:
                xt = sa.tile([PD, B], F32, name="xt")
                nc.sync.dma_start(xt[:], XT[:])
                pt = sa.tile([PD, NU], F32, name="pt")
                nc.sync.dma_start(pt[:], PT[:])
                for uc in range(16):
                    for bt in range(4):
                        p = ps.tile([128, 512], F32, tag="p", name="pA")
                        nc.tensor.matmul(p[:], pt[:, uc * 128:(uc + 1) * 128],
                                         xt[:, bt * 512:(bt + 1) * 512], start=True, stop=True)
                        th = saw.tile([128, 512], F16, tag="ah", name="ah")
                        nc.scalar.copy(th[:], p[:])                       # hi = fp16(prior)
                        tl32 = saw.tile([128, 512], F32, tag="al32", name="al32")
                        nc.vector.tensor_tensor(tl32[:], p[:], th[:], ALU.subtract)
                        tl = saw.tile([128, 512], F16, tag="al", name="al")
                        nc.vector.tensor_copy(tl[:], tl32[:])             # lo = fp16(prior - hi)
                        nc.sync.dma_start(PRH[uc * 128:(uc + 1) * 128, bt * 512:(bt + 1) * 512], th[:])
                        nc.sync.dma_start(PRL[uc * 128:(uc + 1) * 128, bt * 512:(bt + 1) * 512], tl[:])

            # ---------- Stage C: own-row top/mid top-24 ----------
            # Setup runs before stage B; the per-(rb,list) units are emitted
            # interleaved between early stage-B iterations (CPOINTS) so their
            # PSUM-copy/top-k latency hides under stage B's matmul stream.
            sccm = tc.tile_pool(name="stC", bufs=1)
            scp = sccm.__enter__()
            scwcm = tc.tile_pool(name="stCw", bufs=2)
            scw = scwcm.__enter__()
            xo = scp.tile([PD, 256], F32, name="xo")
            nc.sync.dma_start(xo[:], XO[:])
            wsp = scp.tile([PD, LAT], F32, name="wsp")
            nc.sync.dma_start(wsp[:], WSP[:])
            wmp = scp.tile([PD, LAT], F32, name="wmp")
            nc.sync.dma_start(wmp[:], WMP[:])
            wsd = scp.tile([LAT, NT], F32, name="wsd")
            nc.sync.dma_start(wsd[:], WSD[:])
            wmd = scp.tile([LAT, NM], F32, name="wmd")
            nc.sync.dma_start(wmd[:], WMD[:])

            ph1 = ps.tile([128, 512], F32, tag="p", name="ph1")
            nc.tensor.matmul(ph1[:, :256], wsp[:], xo[:], start=True, stop=True)
            h1t = scp.tile([LAT, 256], F32, name="h1t")
            nc.scalar.copy(h1t[:], ph1[:, :256])
            ph2 = ps.tile([128, 512], F32, tag="p", name="ph2")
            nc.tensor.matmul(ph2[:, :256], wmp[:], xo[:], start=True, stop=True)
            h1m = scp.tile([LAT, 256], F32, name="h1m")
            nc.scalar.copy(h1m[:], ph2[:, :256])

            def c_top(rb):
                tsub = scw.tile([128, NT], F32, tag="sub", name="tsub")
                for nt in range(4):
                    w = min(512, NT - nt * 512)
                    p = ps.tile([128, 512], F32, tag="p", name="pC")
                    nc.tensor.matmul(p[:, :w], h1t[:, rb * 128:(rb + 1) * 128],
                                     wsd[:, nt * 512: nt * 512 + w], start=True, stop=True)
                    nc.scalar.copy(tsub[:, nt * 512: nt * 512 + w], p[:, :w])
                _, ixt = _topk24(nc, tk, tsub, NT)
                nc.sync.dma_start(TPOS[rb * 128:(rb + 1) * 128, :], ixt[:])

            def c_mid(rb, mh):
                msub = scw.tile([128, MH], F32, tag="sub", name="msub")
                for nt in range(5):
                    w = min(512, MH - nt * 512)
                    p = ps.tile([128, 512], F32, tag="p", name="pM")
                    nc.tensor.matmul(p[:, :w], h1m[:, rb * 128:(rb + 1) * 128],
                                     wmd[:, mh * MH + nt * 512: mh * MH + nt * 512 + w],
                                     start=True, stop=True)
                    nc.scalar.copy(msub[:, nt * 512: nt * 512 + w], p[:, :w])
                vm, ixm = _topk24(nc, tk, msub, MH)
                nc.sync.dma_start(MPOS[rb * 128:(rb + 1) * 128, mh * NCAND:(mh + 1) * NCAND], ixm[:])
                nc.sync.dma_start(MVAL[rb * 128:(rb + 1) * 128, mh * NCAND:(mh + 1) * NCAND], vm[:])

            C_UNITS = [lambda: c_top(0), lambda: c_mid(0, 0), lambda: c_mid(0, 1),
                       lambda: c_top(1), lambda: c_mid(1, 0), lambda: c_mid(1, 1)]
            CPOINTS = {1: 0, 3: 1, 5: 2, 7: 3, 9: 4, 11: 5}

            def emit_c(i):
                C_UNITS[i]()

            # ---------- Stage B: sim scoring (fp16 hi/lo split, 3 matmuls/chunk) + top-24 ----------
            def sim_bc(h, bc, rsh, rsl):
                if True:
                    pbh = sbw.tile([128, 16 * 128], F16, tag="pbh", name="pbh")
                    nc.sync.dma_start(pbh[:], bass.AP(PRH, bc * 128, [[B, 128], [128 * B, 16], [1, 128]]))
                    pbl = sbw.tile([128, 16 * 128], F16, tag="pbl", name="pbl")
                    nc.sync.dma_start(pbl[:], bass.AP(PRL, bc * 128, [[B, 128], [128 * B, 16], [1, 128]]))
                    scores = sbw.tile([128, HALF], F32, tag="scores", name="scores")
                    tiles = ((0, 512), (512, 512), (1024, 226))
                    pts = [ps.tile([128, 512], F32, tag="p", name="pB") for _ in tiles]
                    # uc outermost so consecutive matmuls share one stationary operand
                    # (pbh[uc] x6, pbl[uc] x3); per-PSUM-tile accumulation order is
                    # unchanged (hh, hl, lh per uc) so results stay bit-identical.
                    for uc in range(16):
                        first = uc == 0
                        last = uc == 15
                        for (nt, w), p in zip(tiles, pts):
                            nc.tensor.matmul(p[:, :w], pbh[:, uc * 128:(uc + 1) * 128],
                                             rsh[uc][:, nt: nt + w], start=first, stop=False)
                            nc.tensor.matmul(p[:, :w], pbh[:, uc * 128:(uc + 1) * 128],
                                             rsl[uc][:, nt: nt + w], start=False, stop=False)
                        for (nt, w), p in zip(tiles, pts):
                            nc.tensor.matmul(p[:, :w], pbl[:, uc * 128:(uc + 1) * 128],
                                             rsh[uc][:, nt: nt + w], start=False, stop=last)
                    for (nt, w), p in zip(tiles, pts):
                        nc.scalar.copy(scores[:, nt:nt + w], p[:, :w])
                    v24, ix24 = _topk24(nc, tk, scores, HALF)
                    idsf = tk.tile([128, NCAND], F32, tag="idsf", name="idsf")
                    nc.vector.tensor_copy(idsf[:], ix24[:])
                    nc.vector.tensor_scalar_add(idsf[:], idsf[:], cb[:, h:h + 1])
                    nc.sync.dma_start(SIMV[bc * 128:(bc + 1) * 128, h * NCAND:(h + 1) * NCAND], v24[:])
                    nc.sync.dma_start(SIMI[bc * 128:(bc + 1) * 128, h * NCAND:(h + 1) * NCAND], idsf[:])

            for bc in range(16):
                sim_bc(0, bc, rshs, rsls)
                if bc in CPOINTS:
                    emit_c(CPOINTS[bc])
            for uc in range(16):
                nc.gpsimd.dma_start(rshs[uc][:], RSH[uc * 128:(uc + 1) * 128, HALF:2 * HALF])
                nc.gpsimd.dma_start(rsls[uc][:], RSL[uc * 128:(uc + 1) * 128, HALF:2 * HALF])
            for bc in range(16):
                sim_bc(1, bc, rshs, rsls)
            scwcm.__exit__(None, None, None)
            sccm.__exit__(None, None, None)
    nc.compile()
    return nc


# ----------------------------------------------------------------------------
# Kernel 2: exact fusion scan (see module docstring of the reference _fuse_one)
# ----------------------------------------------------------------------------
def _build_k2():
    nc = bacc.Bacc(None, target_bir_lowering=False)
    LT = nc.dram_tensor("LT", [256, K], F32, kind="ExternalInput")
    LM = nc.dram_tensor("LM", [256, K], F32, kind="ExternalInput")
    LS = nc.dram_tensor("LS", [256, K], F32, kind="ExternalInput")
    LOGP = nc.dram_tensor("LOGP", [256, 3], F32, kind="ExternalInput")
    GG = nc.dram_tensor("GG", [256, 60], F32, kind="ExternalInput")
    CON = nc.dram_tensor("CON", [128, 200], F32, kind="ExternalInput")
    OUT = nc.dram_tensor("OUT", [256, K], F32, kind="ExternalOutput")

    def v(tl, dims, off=0):
        base = tl[:]
        return bass.AP(base.tensor, base.offset + off, [list(base.ap[0])] + [list(d) for d in dims])

    with tile.TileContext(nc) as tc:
        with tc.tile_pool(name="p", bufs=1) as P:
            def T(w, name):
                return P.tile([128, w], F32, tag=name, name=name)

            t = T(40, "t"); m = T(40, "m"); s = T(40, "s")
            for blk in range(2):
                nc.sync.dma_start(t[:, blk * 20:(blk + 1) * 20], LT[blk * 128:(blk + 1) * 128, :])
                nc.sync.dma_start(m[:, blk * 20:(blk + 1) * 20], LM[blk * 128:(blk + 1) * 128, :])
                nc.sync.dma_start(s[:, blk * 20:(blk + 1) * 20], LS[blk * 128:(blk + 1) * 128, :])
            lp = T(6, "lp")
            for blk in range(2):
                nc.sync.dma_start(lp[:, blk * 3:(blk + 1) * 3], LOGP[blk * 128:(blk + 1) * 128, :])
            g = T(120, "g")
            for blk in range(2):
                nc.sync.dma_start(g[:, blk * 60:(blk + 1) * 60], GG[blk * 128:(blk + 1) * 128, :])
            con = T(200, "con")
            nc.sync.dma_start(con[:], CON[:])

            eq800 = T(800, "eq800")

            def isin(out, a, b, bmask=None):
                eq_v = v(eq800, [[400, 2], [20, 20], [1, 20]])
                a_v = v(a, [[20, 2], [1, 20], [0, 20]])
                b_v = v(b, [[20, 2], [0, 20], [1, 20]])
                nc.vector.tensor_tensor(eq_v, a_v, b_v, ALU.is_equal)
                if bmask is not None:
                    bm_v = v(bmask, [[20, 2], [0, 20], [1, 20]])
                    nc.vector.tensor_tensor(eq_v, eq_v, bm_v, ALU.mult)
                nc.vector.tensor_reduce(v(out, [[20, 2], [1, 20]]),
                                        v(eq800, [[400, 2], [20, 20], [1, 20]]), AX.X, ALU.max)

            def tnot(out, a):
                nc.vector.tensor_scalar(out[:], a[:], -1.0, 1.0, ALU.mult, ALU.add)

            def tmul(out, a, b):
                nc.vector.tensor_tensor(out[:], a[:], b[:], ALU.mult)

            mtm = T(40, "mtm"); mts = T(40, "mts"); mms = T(40, "mms")
            isin(mtm, t, m); isin(mts, t, s); isin(mms, m, s)
            mc = T(40, "mc")
            tmul(mc, mtm, mts); tmul(mc, mc, mms)
            icm = T(40, "icm")
            isin(icm, m, t, bmask=mc)
            ntc = T(40, "ntc"); tnot(ntc, mc)
            mtm2 = T(40, "mtm2"); tmul(mtm2, mtm, ntc)
            mts2 = T(40, "mts2"); tmul(mts2, mts, ntc)
            nicm = T(40, "nicm"); tnot(nicm, icm)
            mms2 = T(40, "mms2"); tmul(mms2, mms, nicm)
            tmp = T(40, "tmpa"); tmp2 = T(40, "tmpb")
            tpm = T(40, "tpm")
            tnot(tmp, mtm2); tnot(tmp2, mts2)
            tmul(tpm, ntc, tmp); tmul(tpm, tpm, tmp2)
            mpm = T(40, "mpm")
            isin(tmp, m, t, bmask=mtm2)
            tnot(tmp, tmp); tnot(tmp2, mms2)
            tmul(mpm, nicm, tmp); tmul(mpm, mpm, tmp2)
            spm = T(40, "spm")
            isin(tmp, s, t, bmask=mc); tnot(tmp, tmp)
            isin(tmp2, s, t, bmask=mts2); tnot(tmp2, tmp2)
            tmul(spm, tmp, tmp2)
            isin(tmp, s, m, bmask=mms2); tnot(tmp, tmp)
            tmul(spm, spm, tmp)

            # ---- det list ----
            dv = T(160, "dv"); dm = T(160, "dm")
            for j, srcm in enumerate((t, t, t, m)):
                nc.vector.tensor_copy(v(dv, [[80, 2], [1, 20]], j * 20), v(srcm, [[20, 2], [1, 20]]))
            for j, srcm in enumerate((mc, mtm2, mts2, mms2)):
                nc.vector.tensor_copy(v(dm, [[80, 2], [1, 20]], j * 20), v(srcm, [[20, 2], [1, 20]]))
            zz = T(80, "zz"); nc.vector.memset(zz[:], 0.0)
            cs = T(160, "cs")
            for blk in range(2):
                nc.vector.tensor_tensor_scan(cs[:, blk * 80:(blk + 1) * 80],
                                             dm[:, blk * 80:(blk + 1) * 80], zz[:],
                                             0.0, ALU.add, ALU.add)
            nd = T(2, "nd")
            nc.vector.tensor_copy(v(nd, [[1, 2]]), v(cs, [[80, 2]], 79))
            pos = T(160, "pos"); csm1 = T(160, "csm1")
            nc.vector.tensor_tensor(pos[:], v(con, [[0, 2], [1, 80]], 20), v(cs, [[80, 2], [1, 80]]), ALU.subtract)
            nc.vector.tensor_tensor(pos[:], pos[:], v(nd, [[1, 2], [0, 80]]), ALU.add)
            nc.vector.tensor_scalar(csm1[:], cs[:], -1.0, None, ALU.add)
            nc.vector.tensor_tensor(csm1[:], csm1[:], pos[:], ALU.subtract)
            nc.vector.tensor_tensor(csm1[:], csm1[:], dm[:], ALU.mult)
            nc.vector.tensor_tensor(pos[:], pos[:], csm1[:], ALU.add)
            E = T(3200, "E")
            E_v = v(E, [[1600, 2], [80, 20], [1, 80]])
            nc.vector.tensor_tensor(E_v, v(con, [[0, 2], [1, 20], [0, 80]], 0),
                                    v(pos, [[80, 2], [0, 20], [1, 80]]), ALU.is_equal)
            nc.vector.tensor_tensor(E_v, E_v, v(dv, [[80, 2], [0, 20], [1, 80]]), ALU.mult)
            det20 = T(40, "det20")
            nc.vector.tensor_reduce(v(det20, [[20, 2], [1, 20]]),
                                    v(E, [[1600, 2], [80, 20], [1, 80]]), AX.X, ALU.add)

            # ---- pools ----
            pv = T(120, "pv"); pm = T(120, "pm")
            for li, srcm in enumerate((t, m, s)):
                nc.vector.tensor_copy(v(pv, [[60, 2], [1, 20]], li * 20), v(srcm, [[20, 2], [1, 20]]))
            for li, srcm in enumerate((tpm, mpm, spm)):
                nc.vector.tensor_copy(v(pm, [[60, 2], [1, 20]], li * 20), v(srcm, [[20, 2], [1, 20]]))
            csp = T(120, "csp")
            for bl in range(6):
                nc.vector.tensor_tensor_scan(csp[:, bl * 20:(bl + 1) * 20],
                                             pm[:, bl * 20:(bl + 1) * 20], zz[:, :20],
                                             0.0, ALU.add, ALU.add)
            cnt = T(6, "cnt")
            nc.vector.tensor_copy(v(cnt, [[1, 6]]), v(csp, [[20, 6]], 19))
            pp = T(120, "pp"); cspm1 = T(120, "cspm1")
            nc.vector.tensor_tensor(pp[:], v(con, [[0, 6], [1, 20]], 0), v(csp, [[20, 6], [1, 20]]), ALU.subtract)
            nc.vector.tensor_tensor(pp[:], pp[:], v(cnt, [[1, 6], [0, 20]]), ALU.add)
            nc.vector.tensor_scalar(cspm1[:], csp[:], -1.0, None, ALU.add)
            nc.vector.tensor_tensor(cspm1[:], cspm1[:], pp[:], ALU.subtract)
            nc.vector.tensor_tensor(cspm1[:], cspm1[:], pm[:], ALU.mult)
            nc.vector.tensor_tensor(pp[:], pp[:], cspm1[:], ALU.add)
            E2 = T(2400, "E2")
            E2_v = v(E2, [[400, 6], [20, 20], [1, 20]])
            nc.vector.tensor_tensor(E2_v, v(con, [[0, 6], [1, 20], [0, 20]], 0),
                                    v(pp, [[20, 6], [0, 20], [1, 20]]), ALU.is_equal)
            nc.vector.tensor_tensor(E2_v, E2_v, v(pv, [[20, 6], [0, 20], [1, 20]]), ALU.mult)
            pc = T(120, "pc")
            nc.vector.tensor_reduce(v(pc, [[20, 6], [1, 20]]),
                                    v(E2, [[400, 6], [20, 20], [1, 20]]), AX.X, ALU.add)

            # ---- fusion scan ----
            sct = T(6, "sct")
            ptrA = T(6, "ptrA"); ptrB = T(6, "ptrB")
            nc.vector.memset(ptrA[:], 0.0)
            outb = T(40, "outb")
            av = T(6, "av"); sc2 = T(6, "sc2"); mx = T(2, "mx"); eqm = T(6, "eqm")
            i1 = T(2, "i1"); i2 = T(2, "i2"); idx = T(2, "idx"); anyav = T(2, "anyav")
            oh = T(6, "oh"); pm6 = T(6, "pm6"); psel = T(2, "psel"); tgt = T(2, "tgt")
            sel = T(120, "sel"); sp = T(120, "sp"); samp = T(2, "samp")
            used = T(2, "used"); d1 = T(2, "d1"); d2 = T(2, "d2"); val = T(2, "val")
            ndet = T(2, "ndet"); dp = T(6, "dp")
            for tstep in range(K):
                ptr, ptrn = (ptrA, ptrB) if tstep % 2 == 0 else (ptrB, ptrA)
                nc.vector.tensor_tensor(av[:], ptr[:], cnt[:], ALU.is_lt)
                nc.vector.tensor_tensor(sc2[:], lp[:], av[:], ALU.mult)
                nc.vector.tensor_scalar(sct[:], av[:], 3.0e38, -3.0e38, ALU.mult, ALU.add)
                nc.vector.tensor_tensor(sc2[:], sc2[:], sct[:], ALU.add)
                nc.vector.tensor_tensor(sc2[:], sc2[:], v(g, [[60, 2], [1, 3]], tstep * 3), ALU.add)
                nc.vector.tensor_reduce(v(mx, [[1, 2]]), v(sc2, [[3, 2], [1, 3]]), AX.X, ALU.max)
                nc.vector.tensor_tensor(eqm[:], sc2[:], v(mx, [[1, 2], [0, 3]]), ALU.is_equal)
                nc.vector.tensor_scalar(i1[:], v(eqm, [[3, 2]], 0), -1.0, 1.0, ALU.mult, ALU.add)
                nc.vector.tensor_scalar(i2[:], v(eqm, [[3, 2]], 1), -1.0, 2.0, ALU.mult, ALU.add)
                nc.vector.tensor_tensor(idx[:], i1[:], i2[:], ALU.mult)
                nc.vector.tensor_reduce(v(anyav, [[1, 2]]), v(av, [[3, 2], [1, 3]]), AX.X, ALU.max)
                nc.vector.tensor_tensor(idx[:], idx[:], anyav[:], ALU.mult)
                nc.vector.tensor_tensor(oh[:], v(con, [[1, 6]], 160), v(idx, [[1, 2], [0, 3]]), ALU.is_equal)
                nc.vector.tensor_tensor(pm6[:], ptr[:], oh[:], ALU.mult)
                nc.vector.tensor_reduce(v(psel, [[1, 2]]), v(pm6, [[3, 2], [1, 3]]), AX.X, ALU.add)
                nc.vector.tensor_scalar(psel[:], psel[:], float(K - 1), None, ALU.min)
                nc.vector.tensor_scalar(tgt[:], idx[:], 20.0, None, ALU.mult)
                nc.vector.tensor_tensor(tgt[:], tgt[:], psel[:], ALU.add)
                nc.vector.tensor_tensor(sel[:], v(con, [[0, 2], [1, 60]], 100),
                                        v(tgt, [[1, 2], [0, 60]]), ALU.is_equal)
                nc.vector.tensor_tensor(sp[:], sel[:], pc[:], ALU.mult)
                nc.vector.tensor_reduce(v(samp, [[1, 2]]), v(sp, [[60, 2], [1, 60]]), AX.X, ALU.add)
                nc.vector.tensor_scalar(used[:], nd[:], float(tstep), None, ALU.is_gt)
                nc.vector.tensor_tensor(d1[:], v(det20, [[20, 2]], tstep), samp[:], ALU.subtract)
                nc.vector.tensor_tensor(d2[:], used[:], d1[:], ALU.mult)
                nc.vector.tensor_tensor(val[:], samp[:], d2[:], ALU.add)
                nc.vector.tensor_copy(v(outb, [[20, 2]], tstep), val[:])
                nc.vector.tensor_scalar(ndet[:], used[:], -1.0, 1.0, ALU.mult, ALU.add)
                nc.vector.tensor_tensor(dp[:], oh[:], v(ndet, [[1, 2], [0, 3]]), ALU.mult)
                nc.vector.tensor_tensor(ptrn[:], ptr[:], dp[:], ALU.add)
            for blk in range(2):
                nc.sync.dma_start(OUT[blk * 128:(blk + 1) * 128, :], outb[:, blk * 20:(blk + 1) * 20])
    nc.compile()
    return nc


_K1 = None
_K2 = None


def _run_spmd(nc, in_maps, core_ids):
    """run_bass_kernel_spmd with retries: transient NRT device errors
    (e.g. NRT_EXEC_UNIT_UNRECOVERABLE right after another process released
    the cores) usually succeed on the next attempt."""
    last = None
    for _ in range(3):
        try:
            return run_bass_kernel_spmd(nc, in_maps, core_ids).results
        except Exception as e:   # noqa: BLE001
            last = e
            import time
            time.sleep(2.0)
    raise last


def _get_k1():
    global _K1
    if _K1 is None:
        _K1 = _build_k1()
    return _K1


def _get_k2():
    global _K2
    if _K2 is None:
        _K2 = _build_k2()
    return _K2


def _merge_desc(vals, ids, k):
    """Per-row top-k by value desc, ties broken by ascending id (jax.lax.top_k semantics)."""
    order = np.lexsort((ids, -vals.astype(np.float64)), axis=-1)[:, :k]
    return np.take_along_axis(ids, order, axis=1)


def kernel(**inputs):
    X = np.ascontiguousarray(np.asarray(inputs["X"], np.float32))
    W_sp = np.ascontiguousarray(np.asarray(inputs["W_sp"], np.float32))
    W_sd = np.ascontiguousarray(np.asarray(inputs["W_sd"], np.float32))
    W_mp = np.ascontiguousarray(np.asarray(inputs["W_mp"], np.float32))
    W_md = np.ascontiguousarray(np.asarray(inputs["W_md"], np.float32))
    W_mapper = np.ascontiguousarray(np.asarray(inputs["W_mapper"], np.float32))
    UR = np.ascontiguousarray(np.asarray(inputs["user_ratings"], np.float32))
    UP = np.ascontiguousarray(np.asarray(inputs["user_personalities"], np.float32))
    top_map = np.asarray(inputs["top_map"]).astype(np.int64)
    mid_map = np.asarray(inputs["mid_map"]).astype(np.int64)

    # ---------------- kernel 1 ----------------
    nc1 = _get_k1()
    XT = np.ascontiguousarray(X.T)
    PTp = np.zeros((PD, NU), np.float32)
    PTp[:, :UP.shape[0]] = UP.T
    in_maps1 = []
    URH = UR.astype(np.float16)
    URL = (UR - URH.astype(np.float32)).astype(np.float16)
    for c in range(NCORES):
        XO = np.ascontiguousarray(X[c * 256:(c + 1) * 256, :].T)
        RSh = np.zeros((NU, SLICE), np.float16)
        RSh[:UR.shape[0], :] = URH[:, c * SLICE:(c + 1) * SLICE]
        RSl = np.zeros((NU, SLICE), np.float16)
        RSl[:UR.shape[0], :] = URL[:, c * SLICE:(c + 1) * SLICE]
        cbase = np.empty((128, 2), np.float32)
        cbase[:, 0] = c * SLICE
        cbase[:, 1] = c * SLICE + HALF
        in_maps1.append({
            "XT": XT, "PT": PTp, "XO": XO, "RSH": RSh, "RSL": RSl,
            "WSP": W_sp, "WMP": W_mp, "WSD": W_sd, "WMD": W_md, "CB": cbase,
        })
    r1 = _run_spmd(nc1, in_maps1, list(range(NCORES)))

    # ---------------- host glue ----------------
    # top: positions already global within [0,2000): map through top_map
    tpos = np.concatenate([r1[c]["TPOS"] for c in range(NCORES)], axis=0).astype(np.int64)
    top20 = top_map[tpos[:, :K]].astype(np.float32)
    # mid: merge the two 2500-halves by value
    mpos = np.concatenate([r1[c]["MPOS"] for c in range(NCORES)], axis=0).astype(np.int64)
    mval = np.concatenate([r1[c]["MVAL"] for c in range(NCORES)], axis=0)
    mpos[:, NCAND:] += MH
    mid20 = mid_map[_merge_desc(mval, mpos, K)].astype(np.float32)
    # sim: merge 8 cores x 48 candidates by value
    simv = np.concatenate([r1[c]["SIMV"] for c in range(NCORES)], axis=1)   # [2048, 384]
    simi = np.concatenate([r1[c]["SIMI"] for c in range(NCORES)], axis=1).astype(np.int64)
    sim20 = _merge_desc(simv, simi, K).astype(np.float32)
    # probs / logp (tiny fp32 matmul; same rounding class as the reference's)
    z = X @ W_mapper
    zm = z - z.max(axis=1, keepdims=True)
    e = np.exp(zm)
    probs = (e / e.sum(axis=1, keepdims=True)).astype(np.float32)
    logp = np.log(probs).astype(np.float32)
    G = _gumbel_G(B, K, 42)

    # ---------------- kernel 2 ----------------
    nc2 = _get_k2()
    con = np.zeros((128, 200), np.float32)
    con[:, 0:20] = np.arange(20)
    con[:, 20:100] = np.arange(80)
    con[:, 100:160] = np.arange(60)
    con[:, 160:166] = np.array([0, 1, 2, 0, 1, 2])
    in_maps2 = []
    for c in range(NCORES):
        r = slice(c * 256, (c + 1) * 256)
        in_maps2.append({
            "LT": np.ascontiguousarray(top20[r]),
            "LM": np.ascontiguousarray(mid20[r]),
            "LS": np.ascontiguousarray(sim20[r]),
            "LOGP": np.ascontiguousarray(logp[r]),
            "GG": np.ascontiguousarray(G[r].reshape(256, 60)),
            "CON": con,
        })
    r2 = _run_spmd(nc2, in_maps2, list(range(NCORES)))
    out = np.concatenate([r2[c]["OUT"] for c in range(NCORES)], axis=0)
    return out.astype(np.float32)
